# revision 8
# baseline (speedup 1.0000x reference)
"""Trainium2 Bass kernel for EnhancedMultiHeadAttention (Shaw-style relative
position bias), sharded tensor-parallel over heads across 8 NeuronCores.

Reference computation (B=4, S=1024, E=1024, H=16, D=64, MAX_REL=512):
    Q = q@Wq+bq; K = q@Wk+bk; V = q@Wv+bv          (per head h: D=64 slices)
    scores = QK^T/8 + bias,  bias[i,j] = Q[i]·rel_table[clip(j-i+512,0,1024)]
    out = softmax(scores) @ V @ Wo + bo

Sharding: core c owns heads {2c, 2c+1} = columns [128c, 128c+128) of
Wq/Wk/Wv and rows [128c, 128c+128) of Wo.  Each core computes its partial
out^T = Wo_c^T @ ctx_c  (bf16, [1024, 4096]); host sums the 8 partials,
transposes back and adds bo.

Device-side structure per core (all matmuls bf16, psum fp32):
  - projections Q^T,K^T [128, 4096] (token-transposed) and V [tok, 2*65]
    (natural layout, with a ones-column per head for softmax denominators)
  - per (b, h): P = Q_h @ rel_table^T  ([1024, 1280] window, clamp baked
    into the padded table), written to DRAM with a SHEARED stride (1281)
    and read back with a rectangular stride (1280) => the per-row diagonal
    shift j-i becomes a plain strided DMA.
  - scores^ (natural [i-part, j-free]) = identity-matmul(bias) + QK^T
    accumulated in psum; far-off-diagonal tiles (|j-i|>639, fully clamped)
    get their (per-row constant) bias via the ACT per-partition bias
    operand instead of the DMA.
  - exp via ACT -> bf16, transposed [i,j]->[j,i] by the DMA xbar
    (dma_start_transpose), A@V with V as stationary, denominators from the
    ones-column, normalization on the small ctx^T, then out-projection.
"""

import sys

sys.path.insert(0, "/opt/trn_rl_repo")

from contextlib import ExitStack

import numpy as np
import ml_dtypes

BF = ml_dtypes.bfloat16

B, S, E, H, D = 4, 1024, 1024, 16, 64
TOK = B * S            # 4096
NCORES = 8
HPC = H // NCORES      # heads per core = 2
MAX_REL = 512
W = 1280               # Ppad row width (w = j - i + 640, w in [1, 1279] used)
WS = W + 1             # sheared row stride
BAND = 4               # |block_i - block_j| <= BAND handled via diagonal DMA
NC128 = S // 128       # 8 chunks per sequence

_CACHE = {}


def _build():
    import concourse.bacc as bacc
    import concourse.tile as tile
    from concourse import mybir
    from concourse.ap import AP

    F32 = mybir.dt.float32
    BF16 = mybir.dt.bfloat16
    EXP = mybir.ActivationFunctionType.Exp
    IDENT = mybir.ActivationFunctionType.Identity

    nc = bacc.Bacc(
        "TRN2", target_bir_lowering=False, debug=False, num_devices=NCORES
    )

    # ---------------- DRAM I/O ----------------
    qT_d = nc.dram_tensor("qT", [E, TOK], BF16, kind="ExternalInput")
    wq_d = nc.dram_tensor("wq", [E, 128], BF16, kind="ExternalInput")
    wk_d = nc.dram_tensor("wk", [E, 128], BF16, kind="ExternalInput")
    wv_d = nc.dram_tensor("wv", [E, 128], BF16, kind="ExternalInput")
    wo_d = nc.dram_tensor("wo", [128, E], BF16, kind="ExternalInput")
    bq_d = nc.dram_tensor("bq", [128, 1], F32, kind="ExternalInput")
    bk_d = nc.dram_tensor("bk", [128, 1], F32, kind="ExternalInput")
    bv_d = nc.dram_tensor("bv", [1, 128], BF16, kind="ExternalInput")
    tt_d = nc.dram_tensor("ttT", [128, W], BF16, kind="ExternalInput")
    id_d = nc.dram_tensor("ident", [128, 128], BF16, kind="ExternalInput")
    out_d = nc.dram_tensor("outT", [E, TOK], BF16, kind="ExternalOutput")

    def split512(lo, hi):
        """split [lo,hi) at 512-grid lines (psum bank boundaries)"""
        if lo >= hi:
            return []
        cuts = [lo]
        g = (lo // 512 + 1) * 512
        while g < hi:
            cuts.append(g)
            g += 512
        cuts.append(hi)
        return list(zip(cuts[:-1], cuts[1:]))

    with tile.TileContext(nc) as tc, ExitStack() as ctx:
        const = ctx.enter_context(tc.tile_pool(name="const", bufs=1))
        big = ctx.enter_context(tc.tile_pool(name="bigsb", bufs=1))
        work = ctx.enter_context(tc.tile_pool(name="work", bufs=3))
        atp = ctx.enter_context(tc.tile_pool(name="atp", bufs=2))
        ctxp = ctx.enter_context(tc.tile_pool(name="ctxp", bufs=2))
        psA = ctx.enter_context(tc.tile_pool(name="psA", bufs=2, space="PSUM"))
        psB = ctx.enter_context(tc.tile_pool(name="psB", bufs=1, space="PSUM"))
        dram = ctx.enter_context(tc.tile_pool(name="dram", bufs=3, space="DRAM"))

        # ------------- load constants / inputs -------------
        qT = big.tile([128, 8, TOK], BF16, tag="qT")
        nc.sync.dma_start(qT[:], qT_d.ap().rearrange("(c p) t -> p c t", p=128))
        wq = const.tile([128, 8, 128], BF16, tag="wq")
        nc.sync.dma_start(wq[:], wq_d.ap().rearrange("(c p) m -> p c m", p=128))
        wk = const.tile([128, 8, 128], BF16, tag="wk")
        nc.sync.dma_start(wk[:], wk_d.ap().rearrange("(c p) m -> p c m", p=128))
        wv = const.tile([128, 8, 128], BF16, tag="wv")
        nc.sync.dma_start(wv[:], wv_d.ap().rearrange("(c p) m -> p c m", p=128))
        wo = const.tile([128, E], BF16, tag="wo")
        nc.sync.dma_start(wo[:], wo_d.ap())
        bq = const.tile([128, 1], F32, tag="bq")
        nc.sync.dma_start(bq[:], bq_d.ap())
        bk = const.tile([128, 1], F32, tag="bk")
        nc.sync.dma_start(bk[:], bk_d.ap())
        bv = const.tile([1, 128], BF16, tag="bv")
        nc.sync.dma_start(bv[:], bv_d.ap())
        ttT = const.tile([128, W], BF16, tag="ttT")
        nc.sync.dma_start(ttT[:], tt_d.ap())
        ident = const.tile([128, 128], BF16, tag="ident")
        nc.sync.dma_start(ident[:], id_d.ap())
        ones1 = const.tile([1, 128], BF16, tag="ones1")
        nc.vector.memset(ones1[:], 1.0)
        onesF = const.tile([128, 64], F32, tag="onesF")
        nc.vector.memset(onesF[:], 1.0)

        QT = big.tile([128, TOK], BF16, tag="QT")
        KT = big.tile([128, TOK], BF16, tag="KT")
        V = big.tile([128, 32, 130], BF16, tag="V")
        nc.vector.memset(V[:, :, 64:65], 1.0)
        nc.vector.memset(V[:, :, 129:130], 1.0)

        # ------------- projections -------------
        # Q^T, K^T: [128(e_out), TOK] = W^T q^T, bias added via ACT
        for dst, wgt, bias in ((QT, wq, bq), (KT, wk, bk)):
            for t8 in range(8):
                ps = psA.tile([128, 512], F32, tag="big")
                for ec in range(8):
                    nc.tensor.matmul(
                        ps[:],
                        wgt[:, ec, :],
                        qT[:, ec, t8 * 512:(t8 + 1) * 512],
                        start=(ec == 0),
                        stop=(ec == 7),
                    )
                nc.scalar.activation(
                    dst[:, t8 * 512:(t8 + 1) * 512], ps[:], IDENT,
                    bias=bias[:], scale=1.0,
                )
        # V natural: [tok(128-chunk), 130] ; cols 0:64 h0, 65:129 h1,
        # ones at 64 and 129 (memset above)
        for tcx in range(32):
            ps = psA.tile([128, 128], F32, tag="big")
            for ec in range(8):
                nc.tensor.matmul(
                    ps[:],
                    qT[:, ec, tcx * 128:(tcx + 1) * 128],
                    wv[:, ec, :],
                    start=(ec == 0),
                    stop=False,
                )
            nc.tensor.matmul(ps[:], ones1[:], bv[:], start=False, stop=True)
            nc.vector.tensor_copy(V[:, tcx, 0:64], ps[:, 0:64])
            nc.vector.tensor_copy(V[:, tcx, 65:129], ps[:, 64:128])

        # ------------- attention per (b, h) -------------
        for b in range(B):
            t0 = b * S
            ctxs = ctxp.tile([128, S], BF16, tag="ctxs")
            for h in range(HPC):
                hr0, hr1 = h * 64, h * 64 + 64

                # ---- P = Q_h @ T_ext^T -> DRAM (sheared) ----
                pd = dram.tile([S * WS], BF16, tag="pshear")
                fl = pd[:]
                edges_all = work.tile([128, 8, 2], F32, tag="edges")
                for icc in range(NC128):
                    i0 = icc * 128
                    psP = psA.tile([128, W], F32, tag="big")
                    lhs = QT[hr0:hr1, t0 + i0:t0 + i0 + 128]
                    for lo, hi in split512(0, W):
                        nc.tensor.matmul(
                            psP[:, lo:hi], lhs, ttT[hr0:hr1, lo:hi],
                            start=True, stop=True,
                        )
                    # clamp-edge columns (u=0 at w=128, u=1024 at w=1152)
                    nc.vector.tensor_copy(
                        edges_all[:, icc, 0:1], psP[:, 128:129]
                    )
                    nc.vector.tensor_copy(
                        edges_all[:, icc, 1:2], psP[:, 1152:1153]
                    )
                    pp = work.tile([128, W], BF16, tag="ppad")
                    if icc % 2 == 0:
                        nc.vector.tensor_copy(pp[:], psP[:])
                    else:
                        nc.scalar.copy(pp[:], psP[:])
                    nc.scalar.dma_start(
                        AP(fl.tensor, fl.offset + i0 * WS, [(WS, 128), (1, W)]),
                        pp[:],
                    )

                # ---- scores + exp + transpose ----
                attnT = atp.tile([128, 8, S], BF16, tag="attnT")
                for icc in range(NC128):
                    i0 = icc * 128
                    jlo = max(0, icc - BAND) * 128
                    jhi = min(NC128, icc + BAND + 1) * 128
                    jw = jhi - jlo

                    bias_t = work.tile([128, 9 * 128], BF16, tag="bias")
                    nc.scalar.dma_start(
                        bias_t[:, 0:jw],
                        AP(
                            fl.tensor,
                            fl.offset + i0 * W + jlo + W // 2,
                            [(W, 128), (1, jw)],
                        ),
                    )

                    ps = psA.tile([128, S], F32, tag="big")
                    # band: bias first (start=True), then QK accumulates
                    for lo, hi in split512(jlo, jhi):
                        nc.tensor.matmul(
                            ps[:, lo:hi], ident[:],
                            bias_t[:, lo - jlo:hi - jlo],
                            start=True, stop=False,
                        )
                    lhs = QT[hr0:hr1, t0 + i0:t0 + i0 + 128]
                    for lo, hi in split512(jlo, jhi):
                        nc.tensor.matmul(
                            ps[:, lo:hi], lhs, KT[hr0:hr1, t0 + lo:t0 + hi],
                            start=False, stop=True,
                        )
                    # far tiles: pure QK (bias is a per-row constant -> exp)
                    for flo, fhi in ((0, jlo), (jhi, S)):
                        for lo, hi in split512(flo, fhi):
                            nc.tensor.matmul(
                                ps[:, lo:hi], lhs, KT[hr0:hr1, t0 + lo:t0 + hi],
                                start=True, stop=True,
                            )

                    ex = work.tile([128, S], BF16, tag="exp")
                    if jlo > 0:
                        nc.scalar.activation(
                            ex[:, 0:jlo], ps[:, 0:jlo], EXP,
                            bias=edges_all[:, icc, 0:1], scale=1.0,
                        )
                    nc.scalar.activation(
                        ex[:, jlo:jhi], ps[:, jlo:jhi], EXP, bias=0.0, scale=1.0
                    )
                    if jhi < S:
                        nc.scalar.activation(
                            ex[:, jhi:S], ps[:, jhi:S], EXP,
                            bias=edges_all[:, icc, 1:2], scale=1.0,
                        )
                    # [i, j] -> [j, i] via DMA xbar; attnT[jp, jc, i]
                    nc.sync.dma_start_transpose(
                        attnT[:, :, i0:i0 + 128], ex[:]
                    )

                # ---- A@V (+ denominators via ones column) ----
                psc = psB.tile([65, S], F32, tag="ctx")
                for jc in range(NC128):
                    lhsv = V[:, b * 8 + jc, h * 65:h * 65 + 65]
                    for lo, hi in split512(0, S):
                        nc.tensor.matmul(
                            psc[:, lo:hi], lhsv, attnT[:, jc, lo:hi],
                            start=(jc == 0), stop=(jc == 7),
                        )

                # ---- normalize: ctx[0:64] * (1/rowsum) ----
                # (engine lanes are partition-locked: recip runs on lane 64,
                #  then PE broadcasts the row to partitions 0-63 via a
                #  ones-stationary K=1 matmul)
                recS = work.tile([65, S], F32, tag="recS")
                nc.vector.reciprocal(recS[64:65, :], psc[64:65, :])
                psr = psA.tile([64, S], F32, tag="big")
                for lo, hi in split512(0, S):
                    nc.tensor.matmul(
                        psr[:, lo:hi], onesF[64:65, :], recS[64:65, lo:hi],
                        start=True, stop=True,
                    )
                rbc = work.tile([64, S], F32, tag="rbc")
                nc.vector.tensor_copy(rbc[:], psr[:])
                if h == 0:
                    nc.vector.tensor_mul(ctxs[0:64, :], psc[0:64, :], rbc[:])
                else:
                    th1 = work.tile([64, S], BF16, tag="th1")
                    nc.vector.tensor_mul(th1[:], psc[0:64, :], rbc[:])
                    nc.sync.dma_start(ctxs[64:128, :], th1[:])

            # ---- out projection: out^T[e, tok] = wo^T @ ctx ----
            for ec in range(8):
                pso = psA.tile([128, S], F32, tag="big")
                for lo, hi in split512(0, S):
                    nc.tensor.matmul(
                        pso[:, lo:hi],
                        wo[:, ec * 128:(ec + 1) * 128],
                        ctxs[:, lo:hi],
                        start=True, stop=True,
                    )
                ob = work.tile([128, S], BF16, tag="outsb")
                nc.scalar.copy(ob[:], pso[:])
                nc.sync.dma_start(
                    out_d.ap()[ec * 128:(ec + 1) * 128, t0:t0 + S], ob[:]
                )

    nc.compile()
    return nc


def _host_prep(q, Wq, bq, Wk, bk, Wv, bv, Wo, bo, rel_table):
    x = np.ascontiguousarray(q.reshape(TOK, E).T).astype(BF)  # [E, TOK]
    ident = np.eye(128, dtype=BF)
    # padded/clamped rel table, transposed: ttT[d, w] = T[clip(w-128,0,1024), d]
    u = np.clip(np.arange(W) - 128, 0, 2 * MAX_REL)
    tt1 = np.ascontiguousarray(rel_table[u].T).astype(BF)  # [64, 1280]
    ttT = np.concatenate([tt1, tt1], axis=0)  # both partition halves
    maps = []
    for c in range(NCORES):
        sl = slice(c * 128, (c + 1) * 128)
        maps.append({
            "qT": x,
            "wq": Wq[:, sl].astype(BF),
            "wk": (Wk[:, sl] / 8.0).astype(BF),
            "wv": Wv[:, sl].astype(BF),
            "wo": Wo[sl, :].astype(BF),
            "bq": bq[sl].reshape(128, 1).astype(np.float32),
            "bk": (bk[sl] / 8.0).reshape(128, 1).astype(np.float32),
            "bv": bv[sl].reshape(1, 128).astype(BF),
            "ttT": ttT,
            "ident": ident,
        })
    return maps


def kernel(q, Wq, bq, Wk, bk, Wv, bv, Wo, bo, rel_table, _trace=False):
    from concourse.bass_utils import run_bass_kernel_spmd

    if "nc" not in _CACHE:
        _CACHE["nc"] = _build()
    nc = _CACHE["nc"]

    in_maps = _host_prep(q, Wq, bq, Wk, bk, Wv, bv, Wo, bo, rel_table)
    res = run_bass_kernel_spmd(
        nc, in_maps, list(range(NCORES)), trace=_trace
    )
    _CACHE["last_results"] = res
    acc = np.zeros((E, TOK), np.float32)
    for r in res.results:
        acc += np.asarray(r["outT"], dtype=np.float32)
    out = acc.T.reshape(B, S, E) + bo.astype(np.float32)
    return out.astype(np.float32)


# revision 11
# speedup vs baseline: 1.0110x; 1.0110x over previous
"""Trainium2 Bass kernel for EnhancedMultiHeadAttention (Shaw-style relative
position bias), sharded tensor-parallel over heads across 8 NeuronCores.

Reference computation (B=4, S=1024, E=1024, H=16, D=64, MAX_REL=512):
    Q = q@Wq+bq; K = q@Wk+bk; V = q@Wv+bv          (per head h: D=64 slices)
    scores = QK^T/8 + bias,  bias[i,j] = Q[i]·rel_table[clip(j-i+512,0,1024)]
    out = softmax(scores) @ V @ Wo + bo

Sharding: core c owns heads {2c, 2c+1} = columns [128c, 128c+128) of
Wq/Wk/Wv and rows [128c, 128c+128) of Wo.  Each core computes its partial
out^T = Wo_c^T @ ctx_c  (bf16, [1024, 4096]); host sums the 8 partials,
transposes back and adds bo.

Device-side structure per core (all matmuls bf16, psum fp32):
  - projections Q^T,K^T [128, 4096] (token-transposed) and V [tok, 2*65]
    (natural layout, with a ones-column per head for softmax denominators)
  - per (b, h): P = Q_h @ rel_table^T  ([1024, 1280] window, clamp baked
    into the padded table), written to DRAM with a SHEARED stride (1281)
    and read back with a rectangular stride (1280) => the per-row diagonal
    shift j-i becomes a plain strided DMA.
  - scores^ (natural [i-part, j-free]) = identity-matmul(bias) + QK^T
    accumulated in psum; far-off-diagonal tiles (|j-i|>639, fully clamped)
    get their (per-row constant) bias via the ACT per-partition bias
    operand instead of the DMA.
  - exp via ACT -> bf16, transposed [i,j]->[j,i] by the DMA xbar
    (dma_start_transpose), A@V with V as stationary, denominators from the
    ones-column, normalization on the small ctx^T, then out-projection.
"""

import sys

sys.path.insert(0, "/opt/trn_rl_repo")

from contextlib import ExitStack

import numpy as np
import ml_dtypes

BF = ml_dtypes.bfloat16

B, S, E, H, D = 4, 1024, 1024, 16, 64
TOK = B * S            # 4096
NCORES = 8
HPC = H // NCORES      # heads per core = 2
MAX_REL = 512
W = 1280               # Ppad row width (w = j - i + 640, w in [1, 1279] used)
WS = W + 1             # sheared row stride
BAND = 4               # |block_i - block_j| <= BAND handled via diagonal DMA
NC128 = S // 128       # 8 chunks per sequence

_CACHE = {}


def _build():
    import concourse.bacc as bacc
    import concourse.tile as tile
    from concourse import mybir
    from concourse.ap import AP

    F32 = mybir.dt.float32
    BF16 = mybir.dt.bfloat16
    EXP = mybir.ActivationFunctionType.Exp
    IDENT = mybir.ActivationFunctionType.Identity

    nc = bacc.Bacc(
        "TRN2", target_bir_lowering=False, debug=False, num_devices=NCORES
    )

    # ---------------- DRAM I/O ----------------
    qT_d = nc.dram_tensor("qT", [E, TOK], BF16, kind="ExternalInput")
    wq_d = nc.dram_tensor("wq", [E, 128], BF16, kind="ExternalInput")
    wk_d = nc.dram_tensor("wk", [E, 128], BF16, kind="ExternalInput")
    wv_d = nc.dram_tensor("wv", [E, 128], BF16, kind="ExternalInput")
    wo_d = nc.dram_tensor("wo", [128, E], BF16, kind="ExternalInput")
    bq_d = nc.dram_tensor("bq", [128, 1], F32, kind="ExternalInput")
    bk_d = nc.dram_tensor("bk", [128, 1], F32, kind="ExternalInput")
    bv_d = nc.dram_tensor("bv", [1, 128], BF16, kind="ExternalInput")
    tt_d = nc.dram_tensor("ttT", [128, W], BF16, kind="ExternalInput")
    id_d = nc.dram_tensor("ident", [128, 128], BF16, kind="ExternalInput")
    out_d = nc.dram_tensor("outT", [E, TOK], BF16, kind="ExternalOutput")

    def split512(lo, hi):
        """split [lo,hi) at 512-grid lines (psum bank boundaries)"""
        if lo >= hi:
            return []
        cuts = [lo]
        g = (lo // 512 + 1) * 512
        while g < hi:
            cuts.append(g)
            g += 512
        cuts.append(hi)
        return list(zip(cuts[:-1], cuts[1:]))

    with tile.TileContext(nc) as tc, ExitStack() as ctx:
        const = ctx.enter_context(tc.tile_pool(name="const", bufs=1))
        big = ctx.enter_context(tc.tile_pool(name="bigsb", bufs=1))
        work = ctx.enter_context(tc.tile_pool(name="work", bufs=3))
        atp = ctx.enter_context(tc.tile_pool(name="atp", bufs=2))
        ctxp = ctx.enter_context(tc.tile_pool(name="ctxp", bufs=2))
        psA = ctx.enter_context(tc.tile_pool(name="psA", bufs=3, space="PSUM"))
        psB = ctx.enter_context(tc.tile_pool(name="psB", bufs=1, space="PSUM"))
        dram = ctx.enter_context(tc.tile_pool(name="dram", bufs=3, space="DRAM"))

        # ------------- load constants / inputs -------------
        qT = big.tile([128, 8, TOK], BF16, tag="qT")
        nc.sync.dma_start(qT[:], qT_d.ap().rearrange("(c p) t -> p c t", p=128))
        wq = const.tile([128, 8, 128], BF16, tag="wq")
        nc.sync.dma_start(wq[:], wq_d.ap().rearrange("(c p) m -> p c m", p=128))
        wk = const.tile([128, 8, 128], BF16, tag="wk")
        nc.sync.dma_start(wk[:], wk_d.ap().rearrange("(c p) m -> p c m", p=128))
        wv = const.tile([128, 8, 128], BF16, tag="wv")
        nc.sync.dma_start(wv[:], wv_d.ap().rearrange("(c p) m -> p c m", p=128))
        wo = const.tile([128, E], BF16, tag="wo")
        nc.sync.dma_start(wo[:], wo_d.ap())
        bq = const.tile([128, 1], F32, tag="bq")
        nc.sync.dma_start(bq[:], bq_d.ap())
        bk = const.tile([128, 1], F32, tag="bk")
        nc.sync.dma_start(bk[:], bk_d.ap())
        bv = const.tile([1, 128], BF16, tag="bv")
        nc.sync.dma_start(bv[:], bv_d.ap())
        ttT = const.tile([128, W], BF16, tag="ttT")
        nc.sync.dma_start(ttT[:], tt_d.ap())
        ident = const.tile([128, 128], BF16, tag="ident")
        nc.sync.dma_start(ident[:], id_d.ap())
        ones1 = const.tile([1, 128], BF16, tag="ones1")
        nc.vector.memset(ones1[:], 1.0)
        onesF = const.tile([128, 64], F32, tag="onesF")
        nc.vector.memset(onesF[:], 1.0)

        QT = big.tile([128, TOK], BF16, tag="QT")
        KT = big.tile([128, TOK], BF16, tag="KT")
        V = big.tile([128, 32, 130], BF16, tag="V")
        nc.vector.memset(V[:, :, 64:65], 1.0)
        nc.vector.memset(V[:, :, 129:130], 1.0)

        # ------------- projections -------------
        # Q^T, K^T: [128(e_out), TOK] = W^T q^T, bias added via ACT
        for dst, wgt, bias in ((QT, wq, bq), (KT, wk, bk)):
            for t8 in range(8):
                ps = psA.tile([128, 512], F32, tag="big")
                for ec in range(8):
                    nc.tensor.matmul(
                        ps[:],
                        wgt[:, ec, :],
                        qT[:, ec, t8 * 512:(t8 + 1) * 512],
                        start=(ec == 0),
                        stop=(ec == 7),
                    )
                nc.scalar.activation(
                    dst[:, t8 * 512:(t8 + 1) * 512], ps[:], IDENT,
                    bias=bias[:], scale=1.0,
                )
        # V natural: [tok(128-chunk), 130] ; cols 0:64 h0, 65:129 h1,
        # ones at 64 and 129 (memset above)
        for tcx in range(32):
            ps = psA.tile([128, 128], F32, tag="big")
            for ec in range(8):
                nc.tensor.matmul(
                    ps[:],
                    qT[:, ec, tcx * 128:(tcx + 1) * 128],
                    wv[:, ec, :],
                    start=(ec == 0),
                    stop=False,
                )
            nc.tensor.matmul(ps[:], ones1[:], bv[:], start=False, stop=True)
            nc.vector.tensor_copy(V[:, tcx, 0:64], ps[:, 0:64])
            nc.vector.tensor_copy(V[:, tcx, 65:129], ps[:, 64:128])

        # ------------- attention per (b, h) -------------
        # software pipeline: P-phase of (b,h) runs one step ahead of the
        # attention phase of the previous (b,h), keeping the PE busy while
        # the P DRAM round-trip for the current step completes.
        def emit_p_phase(b, h):
            t0 = b * S
            hr0, hr1 = h * 64, h * 64 + 64
            pd = dram.tile([S * WS], BF16, tag="pshear")
            fl = pd[:]
            edges_all = work.tile([128, 8, 2], F32, tag="edges")
            for icc in range(NC128):
                i0 = icc * 128
                lhs = QT[hr0:hr1, t0 + i0:t0 + i0 + 128]
                psP1 = psA.tile([128, 1024], F32, tag="big")
                psP2 = psA.tile([128, 256], F32, tag="big")
                for lo, hi in split512(0, 1024):
                    nc.tensor.matmul(psP1[:, lo:hi], lhs, ttT[hr0:hr1, lo:hi],
                                     start=True, stop=True)
                nc.tensor.matmul(psP2[:], lhs, ttT[hr0:hr1, 1024:W],
                                 start=True, stop=True)
                # clamp-edge columns (u=0 at w=128, u=1024 at w=1152)
                nc.vector.tensor_copy(edges_all[:, icc, 0:1], psP1[:, 128:129])
                nc.vector.tensor_copy(edges_all[:, icc, 1:2], psP2[:, 128:129])
                pp = work.tile([128, W], BF16, tag="ppad")
                if icc % 2 == 0:
                    nc.vector.tensor_copy(pp[:, 0:1024], psP1[:])
                    nc.scalar.copy(pp[:, 1024:W], psP2[:])
                else:
                    nc.scalar.copy(pp[:, 0:1024], psP1[:])
                    nc.vector.tensor_copy(pp[:, 1024:W], psP2[:])
                eng = nc.sync if icc % 2 == 0 else nc.scalar
                eng.dma_start(
                    AP(fl.tensor, fl.offset + i0 * WS, [(WS, 128), (1, W)]),
                    pp[:],
                )
            return fl, edges_all

        def emit_attn_phase(b, h, fl, edges_all, ctxs):
            t0 = b * S
            hr0, hr1 = h * 64, h * 64 + 64
            attnT = atp.tile([128, 8, S], BF16, tag="attnT")
            for icc in range(NC128):
                i0 = icc * 128
                jlo = max(0, icc - BAND) * 128
                jhi = min(NC128, icc + BAND + 1) * 128
                jw = jhi - jlo

                bias_t = work.tile([128, 9 * 128], BF16, tag="bias")
                eng = nc.sync if icc % 2 == 0 else nc.scalar
                eng.dma_start(
                    bias_t[:, 0:jw],
                    AP(fl.tensor, fl.offset + i0 * W + jlo + W // 2,
                       [(W, 128), (1, jw)]),
                )

                ps = psA.tile([128, S], F32, tag="big")
                # QK first (start=True) so the PE never waits on the bias
                # DMA chain; the identity-matmul bias accumulates after.
                lhs = QT[hr0:hr1, t0 + i0:t0 + i0 + 128]
                for lo, hi in split512(0, S):
                    nc.tensor.matmul(
                        ps[:, lo:hi], lhs, KT[hr0:hr1, t0 + lo:t0 + hi],
                        start=True, stop=(lo >= jhi or hi <= jlo),
                    )
                for lo, hi in split512(jlo, jhi):
                    nc.tensor.matmul(
                        ps[:, lo:hi], ident[:], bias_t[:, lo - jlo:hi - jlo],
                        start=False, stop=True,
                    )

                ex = work.tile([128, S], BF16, tag="exp")
                if jlo > 0:
                    nc.scalar.activation(
                        ex[:, 0:jlo], ps[:, 0:jlo], EXP,
                        bias=edges_all[:, icc, 0:1], scale=1.0,
                    )
                nc.scalar.activation(
                    ex[:, jlo:jhi], ps[:, jlo:jhi], EXP, bias=0.0, scale=1.0
                )
                if jhi < S:
                    nc.scalar.activation(
                        ex[:, jhi:S], ps[:, jhi:S], EXP,
                        bias=edges_all[:, icc, 1:2], scale=1.0,
                    )
                # [i, j] -> [j, i] via DMA xbar; attnT[jp, jc, i]
                eng2 = nc.scalar if icc % 2 == 0 else nc.sync
                eng2.dma_start_transpose(attnT[:, :, i0:i0 + 128], ex[:])

            # ---- A@V (+ denominators via ones column) ----
            psc = psB.tile([65, S], F32, tag="ctx")
            for jc in range(NC128):
                lhsv = V[:, b * 8 + jc, h * 65:h * 65 + 65]
                for lo, hi in split512(0, S):
                    nc.tensor.matmul(
                        psc[:, lo:hi], lhsv, attnT[:, jc, lo:hi],
                        start=(jc == 0), stop=(jc == 7),
                    )

            # ---- normalize: ctx[0:64] * (1/rowsum) ----
            recS = work.tile([65, S], F32, tag="recS")
            nc.vector.reciprocal(recS[64:65, :], psc[64:65, :])
            psr = psA.tile([64, S], F32, tag="big")
            for lo, hi in split512(0, S):
                nc.tensor.matmul(psr[:, lo:hi], onesF[64:65, :],
                                 recS[64:65, lo:hi], start=True, stop=True)
            rbc = work.tile([64, S], F32, tag="rbc")
            nc.vector.tensor_copy(rbc[:], psr[:])
            if h == 0:
                nc.vector.tensor_mul(ctxs[0:64, :], psc[0:64, :], rbc[:])
            else:
                th1 = work.tile([64, S], BF16, tag="th1")
                nc.vector.tensor_mul(th1[:], psc[0:64, :], rbc[:])
                nc.sync.dma_start(ctxs[64:128, :], th1[:])

        def emit_outproj(b, ctxs):
            t0 = b * S
            for ec in range(8):
                pso = psA.tile([128, S], F32, tag="big")
                for lo, hi in split512(0, S):
                    nc.tensor.matmul(
                        pso[:, lo:hi], wo[:, ec * 128:(ec + 1) * 128],
                        ctxs[:, lo:hi], start=True, stop=True,
                    )
                ob = work.tile([128, S], BF16, tag="outsb")
                nc.scalar.copy(ob[:], pso[:])
                eng = nc.sync if ec % 2 == 0 else nc.scalar
                eng.dma_start(
                    out_d.ap()[ec * 128:(ec + 1) * 128, t0:t0 + S], ob[:]
                )

        phases = [(b, h) for b in range(B) for h in range(HPC)]
        pending = None
        ctxs_by_b = {}
        for b, h in phases:
            state = emit_p_phase(b, h)
            if pending is not None:
                pb, ph, pfl, pedges = pending
                if ph == 0:
                    ctxs_by_b[pb] = ctxp.tile([128, S], BF16, tag="ctxs", name=f"ctxs_{pb}")
                emit_attn_phase(pb, ph, pfl, pedges, ctxs_by_b[pb])
                if ph == 1:
                    emit_outproj(pb, ctxs_by_b.pop(pb))
            pending = (b, h, state[0], state[1])
        pb, ph, pfl, pedges = pending
        if ph == 0:
            ctxs_by_b[pb] = ctxp.tile([128, S], BF16, tag="ctxs", name=f"ctxs_{pb}")
        emit_attn_phase(pb, ph, pfl, pedges, ctxs_by_b[pb])
        emit_outproj(pb, ctxs_by_b.pop(pb))

    nc.compile()
    return nc


def _host_prep(q, Wq, bq, Wk, bk, Wv, bv, Wo, bo, rel_table):
    x = np.ascontiguousarray(q.reshape(TOK, E).T).astype(BF)  # [E, TOK]
    ident = np.eye(128, dtype=BF)
    # padded/clamped rel table, transposed: ttT[d, w] = T[clip(w-128,0,1024), d]
    u = np.clip(np.arange(W) - 128, 0, 2 * MAX_REL)
    tt1 = np.ascontiguousarray(rel_table[u].T).astype(BF)  # [64, 1280]
    ttT = np.concatenate([tt1, tt1], axis=0)  # both partition halves
    maps = []
    for c in range(NCORES):
        sl = slice(c * 128, (c + 1) * 128)
        maps.append({
            "qT": x,
            "wq": Wq[:, sl].astype(BF),
            "wk": (Wk[:, sl] / 8.0).astype(BF),
            "wv": Wv[:, sl].astype(BF),
            "wo": Wo[sl, :].astype(BF),
            "bq": bq[sl].reshape(128, 1).astype(np.float32),
            "bk": (bk[sl] / 8.0).reshape(128, 1).astype(np.float32),
            "bv": bv[sl].reshape(1, 128).astype(BF),
            "ttT": ttT,
            "ident": ident,
        })
    return maps


def kernel(q, Wq, bq, Wk, bk, Wv, bv, Wo, bo, rel_table, _trace=False):
    from concourse.bass_utils import run_bass_kernel_spmd

    if "nc" not in _CACHE:
        _CACHE["nc"] = _build()
    nc = _CACHE["nc"]

    in_maps = _host_prep(q, Wq, bq, Wk, bk, Wv, bv, Wo, bo, rel_table)
    res = run_bass_kernel_spmd(
        nc, in_maps, list(range(NCORES)), trace=_trace
    )
    _CACHE["last_results"] = res
    acc = np.zeros((E, TOK), np.float32)
    for r in res.results:
        acc += np.asarray(r["outT"], dtype=np.float32)
    out = acc.T.reshape(B, S, E) + bo.astype(np.float32)
    return out.astype(np.float32)


# revision 15
# speedup vs baseline: 1.0115x; 1.0005x over previous
"""Trainium2 Bass kernel for EnhancedMultiHeadAttention (Shaw-style relative
position bias), sharded tensor-parallel over heads across 8 NeuronCores.

Reference computation (B=4, S=1024, E=1024, H=16, D=64, MAX_REL=512):
    Q = q@Wq+bq; K = q@Wk+bk; V = q@Wv+bv          (per head h: D=64 slices)
    scores = QK^T/8 + bias,  bias[i,j] = Q[i]·rel_table[clip(j-i+512,0,1024)]
    out = softmax(scores) @ V @ Wo + bo

Sharding: core c owns heads {2c, 2c+1} = columns [128c, 128c+128) of
Wq/Wk/Wv and rows [128c, 128c+128) of Wo.  Each core computes its partial
out^T = Wo_c^T @ ctx_c  (bf16, [1024, 4096]); host sums the 8 partials,
transposes back and adds bo.

Device-side structure per core (all matmuls bf16, psum fp32):
  - projections Q^T,K^T [128, 4096] (token-transposed) and V [tok, 2*65]
    (natural layout, with a ones-column per head for softmax denominators)
  - per (b, h): P = Q_h @ rel_table^T  ([1024, 1280] window, clamp baked
    into the padded table), written to DRAM with a SHEARED stride (1281)
    and read back with a rectangular stride (1280) => the per-row diagonal
    shift j-i becomes a plain strided DMA.
  - scores^ (natural [i-part, j-free]) = identity-matmul(bias) + QK^T
    accumulated in psum; far-off-diagonal tiles (|j-i|>639, fully clamped)
    get their (per-row constant) bias via the ACT per-partition bias
    operand instead of the DMA.
  - exp via ACT -> bf16, transposed [i,j]->[j,i] by the DMA xbar
    (dma_start_transpose), A@V with V as stationary, denominators from the
    ones-column, normalization on the small ctx^T, then out-projection.
"""

import sys

sys.path.insert(0, "/opt/trn_rl_repo")

from contextlib import ExitStack

import numpy as np
import ml_dtypes

BF = ml_dtypes.bfloat16

B, S, E, H, D = 4, 1024, 1024, 16, 64
TOK = B * S            # 4096
NCORES = 8
HPC = H // NCORES      # heads per core = 2
MAX_REL = 512
W = 1280               # Ppad row width (w = j - i + 640, w in [1, 1279] used)
WS = W + 1             # sheared row stride
BAND = 4               # |block_i - block_j| <= BAND handled via diagonal DMA
NC128 = S // 128       # 8 chunks per sequence

_CACHE = {}


def _build():
    import concourse.bacc as bacc
    import concourse.tile as tile
    from concourse import mybir
    from concourse.ap import AP

    F32 = mybir.dt.float32
    BF16 = mybir.dt.bfloat16
    EXP = mybir.ActivationFunctionType.Exp
    IDENT = mybir.ActivationFunctionType.Identity

    nc = bacc.Bacc(
        "TRN2", target_bir_lowering=False, debug=False, num_devices=NCORES
    )

    # ---------------- DRAM I/O ----------------
    qT_d = nc.dram_tensor("qT", [E, TOK], BF16, kind="ExternalInput")
    wq_d = nc.dram_tensor("wq", [E, 128], BF16, kind="ExternalInput")
    wk_d = nc.dram_tensor("wk", [E, 128], BF16, kind="ExternalInput")
    wv_d = nc.dram_tensor("wv", [E, 128], BF16, kind="ExternalInput")
    wo_d = nc.dram_tensor("wo", [128, E], BF16, kind="ExternalInput")
    bq_d = nc.dram_tensor("bq", [128, 1], F32, kind="ExternalInput")
    bk_d = nc.dram_tensor("bk", [128, 1], F32, kind="ExternalInput")
    bv_d = nc.dram_tensor("bv", [128, 1], F32, kind="ExternalInput")
    tt_d = nc.dram_tensor("ttT", [128, W], BF16, kind="ExternalInput")
    id_d = nc.dram_tensor("ident", [128, 128], BF16, kind="ExternalInput")
    out_d = nc.dram_tensor("outT", [E, TOK], BF16, kind="ExternalOutput")

    def split512(lo, hi):
        """split [lo,hi) at 512-grid lines (psum bank boundaries)"""
        if lo >= hi:
            return []
        cuts = [lo]
        g = (lo // 512 + 1) * 512
        while g < hi:
            cuts.append(g)
            g += 512
        cuts.append(hi)
        return list(zip(cuts[:-1], cuts[1:]))

    with tile.TileContext(nc) as tc, ExitStack() as ctx:
        const = ctx.enter_context(tc.tile_pool(name="const", bufs=1))
        big = ctx.enter_context(tc.tile_pool(name="bigsb", bufs=1))
        work = ctx.enter_context(tc.tile_pool(name="work", bufs=3))
        atp = ctx.enter_context(tc.tile_pool(name="atp", bufs=2))
        ctxp = ctx.enter_context(tc.tile_pool(name="ctxp", bufs=2))
        psA = ctx.enter_context(tc.tile_pool(name="psA", bufs=3, space="PSUM"))
        psB = ctx.enter_context(tc.tile_pool(name="psB", bufs=1, space="PSUM"))
        dram = ctx.enter_context(tc.tile_pool(name="dram", bufs=3, space="DRAM"))

        # ------------- load constants / inputs -------------
        qT = big.tile([128, 8, TOK], BF16, tag="qT")
        nc.sync.dma_start(qT[:], qT_d.ap().rearrange("(c p) t -> p c t", p=128))
        wq = const.tile([128, 8, 128], BF16, tag="wq")
        nc.sync.dma_start(wq[:], wq_d.ap().rearrange("(c p) m -> p c m", p=128))
        wk = const.tile([128, 8, 128], BF16, tag="wk")
        nc.sync.dma_start(wk[:], wk_d.ap().rearrange("(c p) m -> p c m", p=128))
        wv = const.tile([128, 8, 128], BF16, tag="wv")
        nc.sync.dma_start(wv[:], wv_d.ap().rearrange("(c p) m -> p c m", p=128))
        wo = const.tile([128, E], BF16, tag="wo")
        nc.sync.dma_start(wo[:], wo_d.ap())
        bq = const.tile([128, 1], F32, tag="bq")
        nc.sync.dma_start(bq[:], bq_d.ap())
        bk = const.tile([128, 1], F32, tag="bk")
        nc.sync.dma_start(bk[:], bk_d.ap())
        bv = const.tile([128, 1], F32, tag="bv")
        nc.sync.dma_start(bv[:], bv_d.ap())
        ttT = const.tile([128, W], BF16, tag="ttT")
        nc.sync.dma_start(ttT[:], tt_d.ap())
        ident = const.tile([128, 128], BF16, tag="ident")
        nc.sync.dma_start(ident[:], id_d.ap())
        onesF = const.tile([128, 64], F32, tag="onesF")
        nc.vector.memset(onesF[:], 1.0)

        QT = big.tile([128, TOK], BF16, tag="QT")
        KT = big.tile([128, TOK], BF16, tag="KT")
        V = big.tile([128, 32, 160], BF16, tag="V")
        nc.vector.memset(V[:, :, 64:65], 1.0)
        nc.vector.memset(V[:, :, 144:145], 1.0)

        # ------------- projections -------------
        # Q^T, K^T: [128(e_out), TOK] = W^T q^T, bias added via ACT
        for dst, wgt, bias in ((QT, wq, bq), (KT, wk, bk)):
            for t8 in range(8):
                ps = psA.tile([128, 512], F32, tag="big")
                for ec in range(8):
                    nc.tensor.matmul(
                        ps[:],
                        wgt[:, ec, :],
                        qT[:, ec, t8 * 512:(t8 + 1) * 512],
                        start=(ec == 0),
                        stop=(ec == 7),
                    )
                nc.scalar.activation(
                    dst[:, t8 * 512:(t8 + 1) * 512], ps[:], IDENT,
                    bias=bias[:], scale=1.0,
                )
        # V: project transposed like Q/K (cheap ldweights), then flip to
        # natural [tok, d] layout with two xbar DMA transposes per head.
        VT = big.tile([128, TOK], BF16, tag="VT")
        for t8 in range(8):
            ps = psA.tile([128, 512], F32, tag="big")
            for ec in range(8):
                nc.tensor.matmul(
                    ps[:], wv[:, ec, :], qT[:, ec, t8 * 512:(t8 + 1) * 512],
                    start=(ec == 0), stop=(ec == 7),
                )
            nc.scalar.activation(
                VT[:, t8 * 512:(t8 + 1) * 512], ps[:], IDENT,
                bias=bv[:], scale=1.0,
            )
        nc.sync.dma_start_transpose(V[:, :, 0:64], VT[0:64, :])
        # HW xbar mishandles a base-partition-64 source: shift head-1 rows
        # down to partition 0 with a plain sb->sb DMA first.
        VT2 = atp.tile([64, TOK], BF16, tag="attnT", name="VT2")
        nc.sync.dma_start(VT2[:], VT[64:128, :])
        nc.scalar.dma_start_transpose(V[:, :, 80:144], VT2[:])

        # ------------- attention per (b, h) -------------
        # software pipeline: P-phase of (b,h) runs one step ahead of the
        # attention phase of the previous (b,h), keeping the PE busy while
        # the P DRAM round-trip for the current step completes.
        def emit_p_phase(b, h):
            t0 = b * S
            hr0, hr1 = h * 64, h * 64 + 64
            pd = dram.tile([S * WS], BF16, tag="pshear")
            fl = pd[:]
            edges_all = work.tile([128, 8, 2], F32, tag="edges")
            for icc in range(NC128):
                i0 = icc * 128
                lhs = QT[hr0:hr1, t0 + i0:t0 + i0 + 128]
                psP1 = psA.tile([128, 1024], F32, tag="big")
                psP2 = psA.tile([128, 256], F32, tag="big")
                for lo, hi in split512(0, 1024):
                    nc.tensor.matmul(psP1[:, lo:hi], lhs, ttT[hr0:hr1, lo:hi],
                                     start=True, stop=True)
                nc.tensor.matmul(psP2[:], lhs, ttT[hr0:hr1, 1024:W],
                                 start=True, stop=True)
                # clamp-edge columns (u=0 at w=128, u=1024 at w=1152)
                nc.vector.tensor_copy(edges_all[:, icc, 0:1], psP1[:, 128:129])
                nc.vector.tensor_copy(edges_all[:, icc, 1:2], psP2[:, 128:129])
                pp = work.tile([128, W], BF16, tag="ppad")
                if icc % 2 == 0:
                    nc.vector.tensor_copy(pp[:, 0:1024], psP1[:])
                    nc.scalar.copy(pp[:, 1024:W], psP2[:])
                else:
                    nc.scalar.copy(pp[:, 0:1024], psP1[:])
                    nc.vector.tensor_copy(pp[:, 1024:W], psP2[:])
                eng = nc.sync if icc % 2 == 0 else nc.scalar
                eng.dma_start(
                    AP(fl.tensor, fl.offset + i0 * WS, [(WS, 128), (1, W)]),
                    pp[:],
                )
            return fl, edges_all

        def emit_attn_phase(b, h, fl, edges_all, ctxs):
            t0 = b * S
            hr0, hr1 = h * 64, h * 64 + 64
            attnT = atp.tile([128, 8, S], BF16, tag="attnT")
            for icc in range(NC128):
                i0 = icc * 128
                jlo = max(0, icc - BAND) * 128
                jhi = min(NC128, icc + BAND + 1) * 128
                jw = jhi - jlo

                bias_t = work.tile([128, 9 * 128], BF16, tag="bias")
                eng = nc.sync if icc % 2 == 0 else nc.scalar
                eng.dma_start(
                    bias_t[:, 0:jw],
                    AP(fl.tensor, fl.offset + i0 * W + jlo + W // 2,
                       [(W, 128), (1, jw)]),
                )

                ps = psA.tile([128, S], F32, tag="big")
                # QK first (start=True) so the PE never waits on the bias
                # DMA chain; the identity-matmul bias accumulates after.
                lhs = QT[hr0:hr1, t0 + i0:t0 + i0 + 128]
                for lo, hi in split512(0, S):
                    nc.tensor.matmul(
                        ps[:, lo:hi], lhs, KT[hr0:hr1, t0 + lo:t0 + hi],
                        start=True, stop=(lo >= jhi or hi <= jlo),
                    )
                for lo, hi in split512(jlo, jhi):
                    nc.tensor.matmul(
                        ps[:, lo:hi], ident[:], bias_t[:, lo - jlo:hi - jlo],
                        start=False, stop=True,
                    )

                ex = work.tile([128, S], BF16, tag="exp")
                if jlo > 0:
                    nc.scalar.activation(
                        ex[:, 0:jlo], ps[:, 0:jlo], EXP,
                        bias=edges_all[:, icc, 0:1], scale=1.0,
                    )
                nc.scalar.activation(
                    ex[:, jlo:jhi], ps[:, jlo:jhi], EXP, bias=0.0, scale=1.0
                )
                if jhi < S:
                    nc.scalar.activation(
                        ex[:, jhi:S], ps[:, jhi:S], EXP,
                        bias=edges_all[:, icc, 1:2], scale=1.0,
                    )
                # [i, j] -> [j, i] via DMA xbar; attnT[jp, jc, i]
                eng2 = nc.scalar if icc % 2 == 0 else nc.sync
                eng2.dma_start_transpose(attnT[:, :, i0:i0 + 128], ex[:])

            # ---- A@V (+ denominators via ones column) ----
            psc = psB.tile([65, S], F32, tag="ctx")
            for jc in range(NC128):
                lhsv = V[:, b * 8 + jc, h * 80:h * 80 + 65]
                for lo, hi in split512(0, S):
                    nc.tensor.matmul(
                        psc[:, lo:hi], lhsv, attnT[:, jc, lo:hi],
                        start=(jc == 0), stop=(jc == 7),
                    )

            # ---- normalize: ctx[0:64] * (1/rowsum) ----
            recS = work.tile([65, S], F32, tag="recS")
            nc.vector.reciprocal(recS[64:65, :], psc[64:65, :])
            psr = psA.tile([64, S], F32, tag="big")
            for lo, hi in split512(0, S):
                nc.tensor.matmul(psr[:, lo:hi], onesF[64:65, :],
                                 recS[64:65, lo:hi], start=True, stop=True)
            rbc = work.tile([64, S], F32, tag="rbc")
            nc.vector.tensor_copy(rbc[:], psr[:])
            if h == 0:
                nc.vector.tensor_mul(ctxs[0:64, :], psc[0:64, :], rbc[:])
            else:
                th1 = work.tile([64, S], BF16, tag="th1")
                nc.vector.tensor_mul(th1[:], psc[0:64, :], rbc[:])
                nc.sync.dma_start(ctxs[64:128, :], th1[:])

        def emit_outproj(b, ctxs):
            t0 = b * S
            for ec in range(8):
                pso = psA.tile([128, S], F32, tag="big")
                for lo, hi in split512(0, S):
                    nc.tensor.matmul(
                        pso[:, lo:hi], wo[:, ec * 128:(ec + 1) * 128],
                        ctxs[:, lo:hi], start=True, stop=True,
                    )
                ob = work.tile([128, S], BF16, tag="outsb")
                nc.scalar.copy(ob[:], pso[:])
                eng = nc.sync if ec % 2 == 0 else nc.scalar
                eng.dma_start(
                    out_d.ap()[ec * 128:(ec + 1) * 128, t0:t0 + S], ob[:]
                )

        phases = [(b, h) for b in range(B) for h in range(HPC)]
        pending = None
        ctxs_by_b = {}
        for b, h in phases:
            state = emit_p_phase(b, h)
            if pending is not None:
                pb, ph, pfl, pedges = pending
                if ph == 0:
                    ctxs_by_b[pb] = ctxp.tile([128, S], BF16, tag="ctxs", name=f"ctxs_{pb}")
                emit_attn_phase(pb, ph, pfl, pedges, ctxs_by_b[pb])
                if ph == 1:
                    emit_outproj(pb, ctxs_by_b.pop(pb))
            pending = (b, h, state[0], state[1])
        pb, ph, pfl, pedges = pending
        if ph == 0:
            ctxs_by_b[pb] = ctxp.tile([128, S], BF16, tag="ctxs", name=f"ctxs_{pb}")
        emit_attn_phase(pb, ph, pfl, pedges, ctxs_by_b[pb])
        emit_outproj(pb, ctxs_by_b.pop(pb))

    nc.compile()
    return nc


def _host_prep(q, Wq, bq, Wk, bk, Wv, bv, Wo, bo, rel_table):
    x = np.ascontiguousarray(q.reshape(TOK, E).T).astype(BF)  # [E, TOK]
    ident = np.eye(128, dtype=BF)
    # padded/clamped rel table, transposed: ttT[d, w] = T[clip(w-128,0,1024), d]
    u = np.clip(np.arange(W) - 128, 0, 2 * MAX_REL)
    tt1 = np.ascontiguousarray(rel_table[u].T).astype(BF)  # [64, 1280]
    ttT = np.concatenate([tt1, tt1], axis=0)  # both partition halves
    maps = []
    for c in range(NCORES):
        sl = slice(c * 128, (c + 1) * 128)
        maps.append({
            "qT": x,
            "wq": Wq[:, sl].astype(BF),
            "wk": (Wk[:, sl] / 8.0).astype(BF),
            "wv": Wv[:, sl].astype(BF),
            "wo": Wo[sl, :].astype(BF),
            "bq": bq[sl].reshape(128, 1).astype(np.float32),
            "bk": (bk[sl] / 8.0).reshape(128, 1).astype(np.float32),
            "bv": bv[sl].reshape(128, 1).astype(np.float32),
            "ttT": ttT,
            "ident": ident,
        })
    return maps


def kernel(q, Wq, bq, Wk, bk, Wv, bv, Wo, bo, rel_table, _trace=False):
    from concourse.bass_utils import run_bass_kernel_spmd

    if "nc" not in _CACHE:
        _CACHE["nc"] = _build()
    nc = _CACHE["nc"]

    in_maps = _host_prep(q, Wq, bq, Wk, bk, Wv, bv, Wo, bo, rel_table)
    res = run_bass_kernel_spmd(
        nc, in_maps, list(range(NCORES)), trace=_trace
    )
    _CACHE["last_results"] = res
    acc = np.zeros((E, TOK), np.float32)
    for r in res.results:
        acc += np.asarray(r["outT"], dtype=np.float32)
    out = acc.T.reshape(B, S, E) + bo.astype(np.float32)
    return out.astype(np.float32)


# revision 16
# speedup vs baseline: 1.0193x; 1.0078x over previous
"""Trainium2 Bass kernel for EnhancedMultiHeadAttention (Shaw-style relative
position bias), sharded tensor-parallel over heads across 8 NeuronCores.

Reference computation (B=4, S=1024, E=1024, H=16, D=64, MAX_REL=512):
    Q = q@Wq+bq; K = q@Wk+bk; V = q@Wv+bv          (per head h: D=64 slices)
    scores = QK^T/8 + bias,  bias[i,j] = Q[i]·rel_table[clip(j-i+512,0,1024)]
    out = softmax(scores) @ V @ Wo + bo

Sharding: core c owns heads {2c, 2c+1} = columns [128c, 128c+128) of
Wq/Wk/Wv and rows [128c, 128c+128) of Wo.  Each core computes its partial
out^T = Wo_c^T @ ctx_c  (bf16, [1024, 4096]); host sums the 8 partials,
transposes back and adds bo.

Device-side structure per core (all matmuls bf16, psum fp32):
  - projections Q^T,K^T [128, 4096] (token-transposed) and V [tok, 2*65]
    (natural layout, with a ones-column per head for softmax denominators)
  - per (b, h): P = Q_h @ rel_table^T  ([1024, 1280] window, clamp baked
    into the padded table), written to DRAM with a SHEARED stride (1281)
    and read back with a rectangular stride (1280) => the per-row diagonal
    shift j-i becomes a plain strided DMA.
  - scores^ (natural [i-part, j-free]) = identity-matmul(bias) + QK^T
    accumulated in psum; far-off-diagonal tiles (|j-i|>639, fully clamped)
    get their (per-row constant) bias via the ACT per-partition bias
    operand instead of the DMA.
  - exp via ACT -> bf16, transposed [i,j]->[j,i] by the DMA xbar
    (dma_start_transpose), A@V with V as stationary, denominators from the
    ones-column, normalization on the small ctx^T, then out-projection.
"""

import sys

sys.path.insert(0, "/opt/trn_rl_repo")

from contextlib import ExitStack

import numpy as np
import ml_dtypes

BF = ml_dtypes.bfloat16

B, S, E, H, D = 4, 1024, 1024, 16, 64
TOK = B * S            # 4096
NCORES = 8
HPC = H // NCORES      # heads per core = 2
MAX_REL = 512
W = 1280               # Ppad row width (w = j - i + 640, w in [1, 1279] used)
WS = W + 1             # sheared row stride
BAND = 4               # |block_i - block_j| <= BAND handled via diagonal DMA
NC128 = S // 128       # 8 chunks per sequence

_CACHE = {}


def _build():
    import concourse.bacc as bacc
    import concourse.tile as tile
    from concourse import mybir
    from concourse.ap import AP

    F32 = mybir.dt.float32
    BF16 = mybir.dt.bfloat16
    EXP = mybir.ActivationFunctionType.Exp
    IDENT = mybir.ActivationFunctionType.Identity

    nc = bacc.Bacc(
        "TRN2", target_bir_lowering=False, debug=False, num_devices=NCORES
    )

    # ---------------- DRAM I/O ----------------
    qT_d = nc.dram_tensor("qT", [E, TOK], BF16, kind="ExternalInput")
    wq_d = nc.dram_tensor("wq", [E, 128], BF16, kind="ExternalInput")
    wk_d = nc.dram_tensor("wk", [E, 128], BF16, kind="ExternalInput")
    wv_d = nc.dram_tensor("wv", [E, 128], BF16, kind="ExternalInput")
    wo_d = nc.dram_tensor("wo", [128, E], BF16, kind="ExternalInput")
    bq_d = nc.dram_tensor("bq", [128, 1], F32, kind="ExternalInput")
    bk_d = nc.dram_tensor("bk", [128, 1], F32, kind="ExternalInput")
    bv_d = nc.dram_tensor("bv", [128, 1], F32, kind="ExternalInput")
    tt_d = nc.dram_tensor("ttT", [128, W], BF16, kind="ExternalInput")
    id_d = nc.dram_tensor("ident", [128, 128], BF16, kind="ExternalInput")
    out_d = nc.dram_tensor("outT", [E, TOK], BF16, kind="ExternalOutput")

    def split512(lo, hi):
        """split [lo,hi) at 512-grid lines (psum bank boundaries)"""
        if lo >= hi:
            return []
        cuts = [lo]
        g = (lo // 512 + 1) * 512
        while g < hi:
            cuts.append(g)
            g += 512
        cuts.append(hi)
        return list(zip(cuts[:-1], cuts[1:]))

    with tile.TileContext(nc) as tc, ExitStack() as ctx:
        const = ctx.enter_context(tc.tile_pool(name="const", bufs=1))
        big = ctx.enter_context(tc.tile_pool(name="bigsb", bufs=1))
        work = ctx.enter_context(tc.tile_pool(name="work", bufs=3))
        atp = ctx.enter_context(tc.tile_pool(name="atp", bufs=2))
        ctxp = ctx.enter_context(tc.tile_pool(name="ctxp", bufs=2))
        psA = ctx.enter_context(tc.tile_pool(name="psA", bufs=3, space="PSUM"))
        psB = ctx.enter_context(tc.tile_pool(name="psB", bufs=2, space="PSUM"))
        dram = ctx.enter_context(tc.tile_pool(name="dram", bufs=3, space="DRAM"))

        # ------------- load constants / inputs -------------
        qT = big.tile([128, 8, TOK], BF16, tag="qT")
        nc.sync.dma_start(qT[:], qT_d.ap().rearrange("(c p) t -> p c t", p=128))
        wq = const.tile([128, 8, 128], BF16, tag="wq")
        nc.sync.dma_start(wq[:], wq_d.ap().rearrange("(c p) m -> p c m", p=128))
        wk = const.tile([128, 8, 128], BF16, tag="wk")
        nc.sync.dma_start(wk[:], wk_d.ap().rearrange("(c p) m -> p c m", p=128))
        wv = const.tile([128, 8, 128], BF16, tag="wv")
        nc.sync.dma_start(wv[:], wv_d.ap().rearrange("(c p) m -> p c m", p=128))
        wo = const.tile([128, E], BF16, tag="wo")
        nc.sync.dma_start(wo[:], wo_d.ap())
        bq = const.tile([128, 1], F32, tag="bq")
        nc.sync.dma_start(bq[:], bq_d.ap())
        bk = const.tile([128, 1], F32, tag="bk")
        nc.sync.dma_start(bk[:], bk_d.ap())
        bv = const.tile([128, 1], F32, tag="bv")
        nc.sync.dma_start(bv[:], bv_d.ap())
        ttT = const.tile([128, W], BF16, tag="ttT")
        nc.sync.dma_start(ttT[:], tt_d.ap())
        ident = const.tile([128, 128], BF16, tag="ident")
        nc.sync.dma_start(ident[:], id_d.ap())
        onesF = const.tile([128, 64], F32, tag="onesF")
        nc.vector.memset(onesF[:], 1.0)

        QT = big.tile([128, TOK], BF16, tag="QT")
        KT = big.tile([128, TOK], BF16, tag="KT")
        V = big.tile([128, 32, 160], BF16, tag="V")
        nc.vector.memset(V[:, :, 64:65], 1.0)
        nc.vector.memset(V[:, :, 144:145], 1.0)

        # ------------- projections -------------
        # Q^T, K^T: [128(e_out), TOK] = W^T q^T, bias added via ACT
        for dst, wgt, bias in ((QT, wq, bq), (KT, wk, bk)):
            for t8 in range(8):
                ps = psA.tile([128, 512], F32, tag="big")
                for ec in range(8):
                    nc.tensor.matmul(
                        ps[:],
                        wgt[:, ec, :],
                        qT[:, ec, t8 * 512:(t8 + 1) * 512],
                        start=(ec == 0),
                        stop=(ec == 7),
                    )
                nc.scalar.activation(
                    dst[:, t8 * 512:(t8 + 1) * 512], ps[:], IDENT,
                    bias=bias[:], scale=1.0,
                )
        # V: project transposed like Q/K (cheap ldweights), then flip to
        # natural [tok, d] layout with two xbar DMA transposes per head.
        VT = big.tile([128, TOK], BF16, tag="VT")
        for t8 in range(8):
            ps = psA.tile([128, 512], F32, tag="big")
            for ec in range(8):
                nc.tensor.matmul(
                    ps[:], wv[:, ec, :], qT[:, ec, t8 * 512:(t8 + 1) * 512],
                    start=(ec == 0), stop=(ec == 7),
                )
            nc.scalar.activation(
                VT[:, t8 * 512:(t8 + 1) * 512], ps[:], IDENT,
                bias=bv[:], scale=1.0,
            )
        nc.sync.dma_start_transpose(V[:, :, 0:64], VT[0:64, :])
        # HW xbar mishandles a base-partition-64 source: shift head-1 rows
        # down to partition 0 with a plain sb->sb DMA first.
        VT2 = atp.tile([64, TOK], BF16, tag="attnT", name="VT2")
        nc.sync.dma_start(VT2[:], VT[64:128, :])
        nc.scalar.dma_start_transpose(V[:, :, 80:144], VT2[:])

        # ------------- attention per (b, h) -------------
        # software pipeline: P-phase of (b,h) runs one step ahead of the
        # attention phase of the previous (b,h), keeping the PE busy while
        # the P DRAM round-trip for the current step completes.
        def emit_p_phase(b, h):
            t0 = b * S
            hr0, hr1 = h * 64, h * 64 + 64
            pd = dram.tile([S * WS], BF16, tag="pshear")
            fl = pd[:]
            edges_all = work.tile([128, 8, 2], F32, tag="edges")
            for icc in range(NC128):
                i0 = icc * 128
                lhs = QT[hr0:hr1, t0 + i0:t0 + i0 + 128]
                psP1 = psA.tile([128, 1024], F32, tag="big")
                psP2 = psA.tile([128, 256], F32, tag="big")
                for lo, hi in split512(0, 1024):
                    nc.tensor.matmul(psP1[:, lo:hi], lhs, ttT[hr0:hr1, lo:hi],
                                     start=True, stop=True)
                nc.tensor.matmul(psP2[:], lhs, ttT[hr0:hr1, 1024:W],
                                 start=True, stop=True)
                # clamp-edge columns (u=0 at w=128, u=1024 at w=1152)
                nc.vector.tensor_copy(edges_all[:, icc, 0:1], psP1[:, 128:129])
                nc.vector.tensor_copy(edges_all[:, icc, 1:2], psP2[:, 128:129])
                pp = work.tile([128, W], BF16, tag="ppad")
                if icc % 2 == 0:
                    nc.vector.tensor_copy(pp[:, 0:1024], psP1[:])
                    nc.scalar.copy(pp[:, 1024:W], psP2[:])
                else:
                    nc.scalar.copy(pp[:, 0:1024], psP1[:])
                    nc.vector.tensor_copy(pp[:, 1024:W], psP2[:])
                nc.gpsimd.dma_start(
                    AP(fl.tensor, fl.offset + i0 * WS, [(WS, 128), (1, W)]),
                    pp[:],
                )
            return fl, edges_all

        def emit_attn_phase(b, h, fl, edges_all, ctxs):
            t0 = b * S
            hr0, hr1 = h * 64, h * 64 + 64
            attnT = atp.tile([128, 8, S], BF16, tag="attnT")
            for icc in range(NC128):
                i0 = icc * 128
                jlo = max(0, icc - BAND) * 128
                jhi = min(NC128, icc + BAND + 1) * 128
                jw = jhi - jlo

                bias_t = work.tile([128, 9 * 128], BF16, tag="bias")
                nc.gpsimd.dma_start(
                    bias_t[:, 0:jw],
                    AP(fl.tensor, fl.offset + i0 * W + jlo + W // 2,
                       [(W, 128), (1, jw)]),
                )

                ps = psA.tile([128, S], F32, tag="big")
                # QK first (start=True) so the PE never waits on the bias
                # DMA chain; the identity-matmul bias accumulates after.
                lhs = QT[hr0:hr1, t0 + i0:t0 + i0 + 128]
                for lo, hi in split512(0, S):
                    nc.tensor.matmul(
                        ps[:, lo:hi], lhs, KT[hr0:hr1, t0 + lo:t0 + hi],
                        start=True, stop=(lo >= jhi or hi <= jlo),
                    )
                for lo, hi in split512(jlo, jhi):
                    nc.tensor.matmul(
                        ps[:, lo:hi], ident[:], bias_t[:, lo - jlo:hi - jlo],
                        start=False, stop=True,
                    )

                ex = work.tile([128, S], BF16, tag="exp")
                if jlo > 0:
                    nc.scalar.activation(
                        ex[:, 0:jlo], ps[:, 0:jlo], EXP,
                        bias=edges_all[:, icc, 0:1], scale=1.0,
                    )
                nc.scalar.activation(
                    ex[:, jlo:jhi], ps[:, jlo:jhi], EXP, bias=0.0, scale=1.0
                )
                if jhi < S:
                    nc.scalar.activation(
                        ex[:, jhi:S], ps[:, jhi:S], EXP,
                        bias=edges_all[:, icc, 1:2], scale=1.0,
                    )
                # [i, j] -> [j, i] via DMA xbar; attnT[jp, jc, i]
                eng2 = nc.scalar if icc % 2 == 0 else nc.sync
                eng2.dma_start_transpose(attnT[:, :, i0:i0 + 128], ex[:])

            # ---- A@V + normalize, in two 512-column halves so the
            # reciprocal chain of one half overlaps the A@V of the next
            # (each half is a single psum bank; psB is double-buffered) ----
            for lo0 in (0, 512):
                hi0 = lo0 + 512
                psc = psB.tile([65, 512], F32, tag="ctx")
                for jc in range(NC128):
                    lhsv = V[:, b * 8 + jc, h * 80:h * 80 + 65]
                    nc.tensor.matmul(
                        psc[:], lhsv, attnT[:, jc, lo0:hi0],
                        start=(jc == 0), stop=(jc == 7),
                    )
                recS = work.tile([65, 512], F32, tag="recS")
                nc.vector.reciprocal(recS[64:65, :], psc[64:65, :])
                psr = psA.tile([64, 512], F32, tag="big")
                nc.tensor.matmul(psr[:], onesF[64:65, :], recS[64:65, :],
                                 start=True, stop=True)
                rbc = work.tile([64, 512], F32, tag="rbc")
                nc.vector.tensor_copy(rbc[:], psr[:])
                if h == 0:
                    nc.vector.tensor_mul(ctxs[0:64, lo0:hi0], psc[0:64, :], rbc[:])
                else:
                    th1 = work.tile([64, 512], BF16, tag="th1")
                    nc.vector.tensor_mul(th1[:], psc[0:64, :], rbc[:])
                    eng = nc.sync if lo0 == 0 else nc.scalar
                    eng.dma_start(ctxs[64:128, lo0:hi0], th1[:])

        def emit_outproj(b, ctxs):
            t0 = b * S
            for ec in range(8):
                pso = psA.tile([128, S], F32, tag="big")
                for lo, hi in split512(0, S):
                    nc.tensor.matmul(
                        pso[:, lo:hi], wo[:, ec * 128:(ec + 1) * 128],
                        ctxs[:, lo:hi], start=True, stop=True,
                    )
                ob = work.tile([128, S], BF16, tag="outsb")
                nc.scalar.copy(ob[:], pso[:])
                eng = nc.sync if ec % 2 == 0 else nc.scalar
                eng.dma_start(
                    out_d.ap()[ec * 128:(ec + 1) * 128, t0:t0 + S], ob[:]
                )

        phases = [(b, h) for b in range(B) for h in range(HPC)]
        pending = None
        ctxs_by_b = {}
        for b, h in phases:
            state = emit_p_phase(b, h)
            if pending is not None:
                pb, ph, pfl, pedges = pending
                if ph == 0:
                    ctxs_by_b[pb] = ctxp.tile([128, S], BF16, tag="ctxs", name=f"ctxs_{pb}")
                emit_attn_phase(pb, ph, pfl, pedges, ctxs_by_b[pb])
                if ph == 1:
                    emit_outproj(pb, ctxs_by_b.pop(pb))
            pending = (b, h, state[0], state[1])
        pb, ph, pfl, pedges = pending
        if ph == 0:
            ctxs_by_b[pb] = ctxp.tile([128, S], BF16, tag="ctxs", name=f"ctxs_{pb}")
        emit_attn_phase(pb, ph, pfl, pedges, ctxs_by_b[pb])
        emit_outproj(pb, ctxs_by_b.pop(pb))

    nc.compile()
    return nc


def _host_prep(q, Wq, bq, Wk, bk, Wv, bv, Wo, bo, rel_table):
    x = np.ascontiguousarray(q.reshape(TOK, E).T).astype(BF)  # [E, TOK]
    ident = np.eye(128, dtype=BF)
    # padded/clamped rel table, transposed: ttT[d, w] = T[clip(w-128,0,1024), d]
    u = np.clip(np.arange(W) - 128, 0, 2 * MAX_REL)
    tt1 = np.ascontiguousarray(rel_table[u].T).astype(BF)  # [64, 1280]
    ttT = np.concatenate([tt1, tt1], axis=0)  # both partition halves
    maps = []
    for c in range(NCORES):
        sl = slice(c * 128, (c + 1) * 128)
        maps.append({
            "qT": x,
            "wq": Wq[:, sl].astype(BF),
            "wk": (Wk[:, sl] / 8.0).astype(BF),
            "wv": Wv[:, sl].astype(BF),
            "wo": Wo[sl, :].astype(BF),
            "bq": bq[sl].reshape(128, 1).astype(np.float32),
            "bk": (bk[sl] / 8.0).reshape(128, 1).astype(np.float32),
            "bv": bv[sl].reshape(128, 1).astype(np.float32),
            "ttT": ttT,
            "ident": ident,
        })
    return maps


def kernel(q, Wq, bq, Wk, bk, Wv, bv, Wo, bo, rel_table, _trace=False):
    from concourse.bass_utils import run_bass_kernel_spmd

    if "nc" not in _CACHE:
        _CACHE["nc"] = _build()
    nc = _CACHE["nc"]

    in_maps = _host_prep(q, Wq, bq, Wk, bk, Wv, bv, Wo, bo, rel_table)
    res = run_bass_kernel_spmd(
        nc, in_maps, list(range(NCORES)), trace=_trace
    )
    _CACHE["last_results"] = res
    acc = np.zeros((E, TOK), np.float32)
    for r in res.results:
        acc += np.asarray(r["outT"], dtype=np.float32)
    out = acc.T.reshape(B, S, E) + bo.astype(np.float32)
    return out.astype(np.float32)


# revision 17
# speedup vs baseline: 1.0477x; 1.0279x over previous
"""Trainium2 Bass kernel for EnhancedMultiHeadAttention (Shaw-style relative
position bias), sharded tensor-parallel over heads across 8 NeuronCores.

Reference computation (B=4, S=1024, E=1024, H=16, D=64, MAX_REL=512):
    Q = q@Wq+bq; K = q@Wk+bk; V = q@Wv+bv          (per head h: D=64 slices)
    scores = QK^T/8 + bias,  bias[i,j] = Q[i]·rel_table[clip(j-i+512,0,1024)]
    out = softmax(scores) @ V @ Wo + bo

Sharding: core c owns heads {2c, 2c+1} = columns [128c, 128c+128) of
Wq/Wk/Wv and rows [128c, 128c+128) of Wo.  Each core computes its partial
out^T = Wo_c^T @ ctx_c  (bf16, [1024, 4096]); host sums the 8 partials,
transposes back and adds bo.

Device-side structure per core (all matmuls bf16, psum fp32):
  - projections Q^T,K^T [128, 4096] (token-transposed) and V [tok, 2*65]
    (natural layout, with a ones-column per head for softmax denominators)
  - per (b, h): P = Q_h @ rel_table^T  ([1024, 1280] window, clamp baked
    into the padded table), written to DRAM with a SHEARED stride (1281)
    and read back with a rectangular stride (1280) => the per-row diagonal
    shift j-i becomes a plain strided DMA.
  - scores^ (natural [i-part, j-free]) = identity-matmul(bias) + QK^T
    accumulated in psum; far-off-diagonal tiles (|j-i|>639, fully clamped)
    get their (per-row constant) bias via the ACT per-partition bias
    operand instead of the DMA.
  - exp via ACT -> bf16, transposed [i,j]->[j,i] by the DMA xbar
    (dma_start_transpose), A@V with V as stationary, denominators from the
    ones-column, normalization on the small ctx^T, then out-projection.
"""

import sys

sys.path.insert(0, "/opt/trn_rl_repo")

from contextlib import ExitStack

import numpy as np
import ml_dtypes

BF = ml_dtypes.bfloat16

B, S, E, H, D = 4, 1024, 1024, 16, 64
TOK = B * S            # 4096
NCORES = 8
HPC = H // NCORES      # heads per core = 2
MAX_REL = 512
W = 1280               # Ppad row width (w = j - i + 640, w in [1, 1279] used)
WS = W + 1             # sheared row stride
BAND = 4               # |block_i - block_j| <= BAND handled via diagonal DMA
NC128 = S // 128       # 8 chunks per sequence

_CACHE = {}


def _build():
    import concourse.bacc as bacc
    import concourse.tile as tile
    from concourse import mybir
    from concourse.ap import AP

    F32 = mybir.dt.float32
    BF16 = mybir.dt.bfloat16
    EXP = mybir.ActivationFunctionType.Exp
    IDENT = mybir.ActivationFunctionType.Identity

    nc = bacc.Bacc(
        "TRN2", target_bir_lowering=False, debug=False, num_devices=NCORES
    )

    # ---------------- DRAM I/O ----------------
    qT_d = nc.dram_tensor("qT", [E, TOK], BF16, kind="ExternalInput")
    wq_d = nc.dram_tensor("wq", [E, 128], BF16, kind="ExternalInput")
    wk_d = nc.dram_tensor("wk", [E, 128], BF16, kind="ExternalInput")
    wv_d = nc.dram_tensor("wv", [E, 128], BF16, kind="ExternalInput")
    wo_d = nc.dram_tensor("wo", [128, E], BF16, kind="ExternalInput")
    bq_d = nc.dram_tensor("bq", [128, 1], F32, kind="ExternalInput")
    bk_d = nc.dram_tensor("bk", [128, 1], F32, kind="ExternalInput")
    bv_d = nc.dram_tensor("bv", [128, 1], F32, kind="ExternalInput")
    tt_d = nc.dram_tensor("ttT", [128, W], BF16, kind="ExternalInput")
    id_d = nc.dram_tensor("ident", [128, 128], BF16, kind="ExternalInput")
    out_d = nc.dram_tensor("outT", [E, TOK], BF16, kind="ExternalOutput")

    def split512(lo, hi):
        """split [lo,hi) at 512-grid lines (psum bank boundaries)"""
        if lo >= hi:
            return []
        cuts = [lo]
        g = (lo // 512 + 1) * 512
        while g < hi:
            cuts.append(g)
            g += 512
        cuts.append(hi)
        return list(zip(cuts[:-1], cuts[1:]))

    with tile.TileContext(nc) as tc, ExitStack() as ctx:
        const = ctx.enter_context(tc.tile_pool(name="const", bufs=1))
        big = ctx.enter_context(tc.tile_pool(name="bigsb", bufs=1))
        work = ctx.enter_context(tc.tile_pool(name="work", bufs=3))
        atp = ctx.enter_context(tc.tile_pool(name="atp", bufs=2))
        ctxp = ctx.enter_context(tc.tile_pool(name="ctxp", bufs=2))
        psA = ctx.enter_context(tc.tile_pool(name="psA", bufs=3, space="PSUM"))
        psB = ctx.enter_context(tc.tile_pool(name="psB", bufs=2, space="PSUM"))
        dram = ctx.enter_context(tc.tile_pool(name="dram", bufs=3, space="DRAM"))

        # ------------- load constants / inputs -------------
        qT = big.tile([128, 8, TOK], BF16, tag="qT")
        nc.sync.dma_start(qT[:], qT_d.ap().rearrange("(c p) t -> p c t", p=128))
        wq = const.tile([128, 8, 128], BF16, tag="wq")
        nc.sync.dma_start(wq[:], wq_d.ap().rearrange("(c p) m -> p c m", p=128))
        wk = const.tile([128, 8, 128], BF16, tag="wk")
        nc.sync.dma_start(wk[:], wk_d.ap().rearrange("(c p) m -> p c m", p=128))
        wv = const.tile([128, 8, 128], BF16, tag="wv")
        nc.sync.dma_start(wv[:], wv_d.ap().rearrange("(c p) m -> p c m", p=128))
        wo = const.tile([128, E], BF16, tag="wo")
        nc.sync.dma_start(wo[:], wo_d.ap())
        bq = const.tile([128, 1], F32, tag="bq")
        nc.sync.dma_start(bq[:], bq_d.ap())
        bk = const.tile([128, 1], F32, tag="bk")
        nc.sync.dma_start(bk[:], bk_d.ap())
        bv = const.tile([128, 1], F32, tag="bv")
        nc.sync.dma_start(bv[:], bv_d.ap())
        ttT = const.tile([128, W], BF16, tag="ttT")
        nc.sync.dma_start(ttT[:], tt_d.ap())
        ident = const.tile([128, 128], BF16, tag="ident")
        nc.sync.dma_start(ident[:], id_d.ap())
        onesF = const.tile([128, 64], F32, tag="onesF")
        nc.vector.memset(onesF[:], 1.0)

        QT = big.tile([128, TOK], BF16, tag="QT")
        KT = big.tile([128, TOK], BF16, tag="KT")
        V = big.tile([128, 32, 160], BF16, tag="V")
        nc.vector.memset(V[:, :, 64:65], 1.0)
        nc.vector.memset(V[:, :, 144:145], 1.0)

        # ------------- projections -------------
        # Q^T, K^T: [128(e_out), TOK] = W^T q^T, bias added via ACT
        for dst, wgt, bias in ((QT, wq, bq), (KT, wk, bk)):
            for t8 in range(8):
                ps = psA.tile([128, 512], F32, tag="big")
                for ec in range(8):
                    nc.tensor.matmul(
                        ps[:],
                        wgt[:, ec, :],
                        qT[:, ec, t8 * 512:(t8 + 1) * 512],
                        start=(ec == 0),
                        stop=(ec == 7),
                    )
                nc.scalar.activation(
                    dst[:, t8 * 512:(t8 + 1) * 512], ps[:], IDENT,
                    bias=bias[:], scale=1.0,
                )
        # V: project transposed like Q/K (cheap ldweights), then flip to
        # natural [tok, d] layout with two xbar DMA transposes per head.
        VT = big.tile([128, TOK], BF16, tag="VT")
        for t8 in range(8):
            ps = psA.tile([128, 512], F32, tag="big")
            for ec in range(8):
                nc.tensor.matmul(
                    ps[:], wv[:, ec, :], qT[:, ec, t8 * 512:(t8 + 1) * 512],
                    start=(ec == 0), stop=(ec == 7),
                )
            nc.scalar.activation(
                VT[:, t8 * 512:(t8 + 1) * 512], ps[:], IDENT,
                bias=bv[:], scale=1.0,
            )
        # bounce V^T through DRAM: DRAM-source xbar transposes avoid the
        # sb->sb-transpose hazard serialization (and read any row offset).
        vtd = dram.tile([128, TOK], BF16, tag="vtd")
        nc.sync.dma_start(vtd[:], VT[:])
        nc.sync.dma_start_transpose(V[:, :, 0:64], vtd[0:64, :])
        nc.scalar.dma_start_transpose(V[:, :, 80:144], vtd[64:128, :])

        # ------------- attention per (b, h) -------------
        # software pipeline: P-phase of (b,h) runs one step ahead of the
        # attention phase of the previous (b,h), keeping the PE busy while
        # the P DRAM round-trip for the current step completes.
        def emit_p_phase(b, h):
            t0 = b * S
            hr0, hr1 = h * 64, h * 64 + 64
            pd = dram.tile([S * WS], BF16, tag="pshear")
            fl = pd[:]
            edges_all = work.tile([128, 8, 2], F32, tag="edges")
            for icc in range(NC128):
                i0 = icc * 128
                lhs = QT[hr0:hr1, t0 + i0:t0 + i0 + 128]
                psP1 = psA.tile([128, 1024], F32, tag="big")
                psP2 = psA.tile([128, 256], F32, tag="big")
                for lo, hi in split512(0, 1024):
                    nc.tensor.matmul(psP1[:, lo:hi], lhs, ttT[hr0:hr1, lo:hi],
                                     start=True, stop=True)
                nc.tensor.matmul(psP2[:], lhs, ttT[hr0:hr1, 1024:W],
                                 start=True, stop=True)
                # clamp-edge columns (u=0 at w=128, u=1024 at w=1152)
                nc.vector.tensor_copy(edges_all[:, icc, 0:1], psP1[:, 128:129])
                nc.vector.tensor_copy(edges_all[:, icc, 1:2], psP2[:, 128:129])
                pp = work.tile([128, W], BF16, tag="ppad")
                if icc % 2 == 0:
                    nc.vector.tensor_copy(pp[:, 0:1024], psP1[:])
                    nc.scalar.copy(pp[:, 1024:W], psP2[:])
                else:
                    nc.scalar.copy(pp[:, 0:1024], psP1[:])
                    nc.vector.tensor_copy(pp[:, 1024:W], psP2[:])
                nc.gpsimd.dma_start(
                    AP(fl.tensor, fl.offset + i0 * WS, [(WS, 128), (1, W)]),
                    pp[:],
                )
            return fl, edges_all

        def emit_attn_phase(b, h, fl, edges_all, ctxs):
            t0 = b * S
            hr0, hr1 = h * 64, h * 64 + 64
            attnT = atp.tile([128, 8, S], BF16, tag="attnT")
            edt = dram.tile([NC128, 128, S], BF16, tag="expd")
            ed = [edt[i] for i in range(NC128)]
            for icc in range(NC128):
                i0 = icc * 128
                jlo = max(0, icc - BAND) * 128
                jhi = min(NC128, icc + BAND + 1) * 128
                jw = jhi - jlo

                bias_t = work.tile([128, 9 * 128], BF16, tag="bias")
                nc.gpsimd.dma_start(
                    bias_t[:, 0:jw],
                    AP(fl.tensor, fl.offset + i0 * W + jlo + W // 2,
                       [(W, 128), (1, jw)]),
                )

                ps = psA.tile([128, S], F32, tag="big")
                # QK first (start=True) so the PE never waits on the bias
                # DMA chain; the identity-matmul bias accumulates after.
                lhs = QT[hr0:hr1, t0 + i0:t0 + i0 + 128]
                for lo, hi in split512(0, S):
                    nc.tensor.matmul(
                        ps[:, lo:hi], lhs, KT[hr0:hr1, t0 + lo:t0 + hi],
                        start=True, stop=(lo >= jhi or hi <= jlo),
                    )
                for lo, hi in split512(jlo, jhi):
                    nc.tensor.matmul(
                        ps[:, lo:hi], ident[:], bias_t[:, lo - jlo:hi - jlo],
                        start=False, stop=True,
                    )

                ex = work.tile([128, S], BF16, tag="exp")
                if jlo > 0:
                    nc.scalar.activation(
                        ex[:, 0:jlo], ps[:, 0:jlo], EXP,
                        bias=edges_all[:, icc, 0:1], scale=1.0,
                    )
                nc.scalar.activation(
                    ex[:, jlo:jhi], ps[:, jlo:jhi], EXP, bias=0.0, scale=1.0
                )
                if jhi < S:
                    nc.scalar.activation(
                        ex[:, jhi:S], ps[:, jhi:S], EXP,
                        bias=edges_all[:, icc, 1:2], scale=1.0,
                    )
                # [i, j] -> [j, i] via DMA xbar, bounced through DRAM
                # (sb->sb xbar transposes are hazard-serialized against all
                # other sb->sb DMA traffic; DRAM-source xbars are not)
                eng2 = nc.scalar if icc % 2 == 0 else nc.sync
                eng3 = nc.sync if icc % 2 == 0 else nc.scalar
                eng2.dma_start(ed[icc], ex[:])
                eng3.dma_start_transpose(attnT[:, :, i0:i0 + 128], ed[icc])

            # ---- A@V + normalize, in two 512-column halves so the
            # reciprocal chain of one half overlaps the A@V of the next
            # (each half is a single psum bank; psB is double-buffered) ----
            for lo0 in (0, 512):
                hi0 = lo0 + 512
                psc = psB.tile([65, 512], F32, tag="ctx")
                for jc in range(NC128):
                    lhsv = V[:, b * 8 + jc, h * 80:h * 80 + 65]
                    nc.tensor.matmul(
                        psc[:], lhsv, attnT[:, jc, lo0:hi0],
                        start=(jc == 0), stop=(jc == 7),
                    )
                recS = work.tile([65, 512], F32, tag="recS")
                nc.vector.reciprocal(recS[64:65, :], psc[64:65, :])
                psr = psA.tile([64, 512], F32, tag="big")
                nc.tensor.matmul(psr[:], onesF[64:65, :], recS[64:65, :],
                                 start=True, stop=True)
                rbc = work.tile([64, 512], F32, tag="rbc")
                nc.vector.tensor_copy(rbc[:], psr[:])
                if h == 0:
                    nc.vector.tensor_mul(ctxs[0:64, lo0:hi0], psc[0:64, :], rbc[:])
                else:
                    th1 = work.tile([64, 512], BF16, tag="th1")
                    nc.vector.tensor_mul(th1[:], psc[0:64, :], rbc[:])
                    eng = nc.sync if lo0 == 0 else nc.scalar
                    eng.dma_start(ctxs[64:128, lo0:hi0], th1[:])

        def emit_outproj(b, ctxs):
            t0 = b * S
            for ec in range(8):
                pso = psA.tile([128, S], F32, tag="big")
                for lo, hi in split512(0, S):
                    nc.tensor.matmul(
                        pso[:, lo:hi], wo[:, ec * 128:(ec + 1) * 128],
                        ctxs[:, lo:hi], start=True, stop=True,
                    )
                ob = work.tile([128, S], BF16, tag="outsb")
                nc.scalar.copy(ob[:], pso[:])
                eng = nc.sync if ec % 2 == 0 else nc.scalar
                eng.dma_start(
                    out_d.ap()[ec * 128:(ec + 1) * 128, t0:t0 + S], ob[:]
                )

        phases = [(b, h) for b in range(B) for h in range(HPC)]
        pending = None
        ctxs_by_b = {}
        for b, h in phases:
            state = emit_p_phase(b, h)
            if pending is not None:
                pb, ph, pfl, pedges = pending
                if ph == 0:
                    ctxs_by_b[pb] = ctxp.tile([128, S], BF16, tag="ctxs", name=f"ctxs_{pb}")
                emit_attn_phase(pb, ph, pfl, pedges, ctxs_by_b[pb])
                if ph == 1:
                    emit_outproj(pb, ctxs_by_b.pop(pb))
            pending = (b, h, state[0], state[1])
        pb, ph, pfl, pedges = pending
        if ph == 0:
            ctxs_by_b[pb] = ctxp.tile([128, S], BF16, tag="ctxs", name=f"ctxs_{pb}")
        emit_attn_phase(pb, ph, pfl, pedges, ctxs_by_b[pb])
        emit_outproj(pb, ctxs_by_b.pop(pb))

    nc.compile()
    return nc


def _host_prep(q, Wq, bq, Wk, bk, Wv, bv, Wo, bo, rel_table):
    x = np.ascontiguousarray(q.reshape(TOK, E).T).astype(BF)  # [E, TOK]
    ident = np.eye(128, dtype=BF)
    # padded/clamped rel table, transposed: ttT[d, w] = T[clip(w-128,0,1024), d]
    u = np.clip(np.arange(W) - 128, 0, 2 * MAX_REL)
    tt1 = np.ascontiguousarray(rel_table[u].T).astype(BF)  # [64, 1280]
    ttT = np.concatenate([tt1, tt1], axis=0)  # both partition halves
    maps = []
    for c in range(NCORES):
        sl = slice(c * 128, (c + 1) * 128)
        maps.append({
            "qT": x,
            "wq": Wq[:, sl].astype(BF),
            "wk": (Wk[:, sl] / 8.0).astype(BF),
            "wv": Wv[:, sl].astype(BF),
            "wo": Wo[sl, :].astype(BF),
            "bq": bq[sl].reshape(128, 1).astype(np.float32),
            "bk": (bk[sl] / 8.0).reshape(128, 1).astype(np.float32),
            "bv": bv[sl].reshape(128, 1).astype(np.float32),
            "ttT": ttT,
            "ident": ident,
        })
    return maps


def kernel(q, Wq, bq, Wk, bk, Wv, bv, Wo, bo, rel_table, _trace=False):
    from concourse.bass_utils import run_bass_kernel_spmd

    if "nc" not in _CACHE:
        _CACHE["nc"] = _build()
    nc = _CACHE["nc"]

    in_maps = _host_prep(q, Wq, bq, Wk, bk, Wv, bv, Wo, bo, rel_table)
    res = run_bass_kernel_spmd(
        nc, in_maps, list(range(NCORES)), trace=_trace
    )
    _CACHE["last_results"] = res
    acc = np.zeros((E, TOK), np.float32)
    for r in res.results:
        acc += np.asarray(r["outT"], dtype=np.float32)
    out = acc.T.reshape(B, S, E) + bo.astype(np.float32)
    return out.astype(np.float32)


# revision 18
# speedup vs baseline: 1.1786x; 1.1249x over previous
"""Trainium2 Bass kernel for EnhancedMultiHeadAttention (Shaw-style relative
position bias), sharded tensor-parallel over heads across 8 NeuronCores.

Reference computation (B=4, S=1024, E=1024, H=16, D=64, MAX_REL=512):
    Q = q@Wq+bq; K = q@Wk+bk; V = q@Wv+bv          (per head h: D=64 slices)
    scores = QK^T/8 + bias,  bias[i,j] = Q[i]·rel_table[clip(j-i+512,0,1024)]
    out = softmax(scores) @ V @ Wo + bo

Sharding: core c owns heads {2c, 2c+1} = columns [128c, 128c+128) of
Wq/Wk/Wv and rows [128c, 128c+128) of Wo.  Each core computes its partial
out^T = Wo_c^T @ ctx_c  (bf16, [1024, 4096]); host sums the 8 partials,
transposes back and adds bo.

Device-side structure per core (all matmuls bf16, psum fp32):
  - projections Q^T,K^T [128, 4096] (token-transposed) and V [tok, 2*65]
    (natural layout, with a ones-column per head for softmax denominators)
  - per (b, h): P = Q_h @ rel_table^T  ([1024, 1280] window, clamp baked
    into the padded table), written to DRAM with a SHEARED stride (1281)
    and read back with a rectangular stride (1280) => the per-row diagonal
    shift j-i becomes a plain strided DMA.
  - scores^ (natural [i-part, j-free]) = identity-matmul(bias) + QK^T
    accumulated in psum; far-off-diagonal tiles (|j-i|>639, fully clamped)
    get their (per-row constant) bias via the ACT per-partition bias
    operand instead of the DMA.
  - exp via ACT -> bf16, transposed [i,j]->[j,i] by the DMA xbar
    (dma_start_transpose), A@V with V as stationary, denominators from the
    ones-column, normalization on the small ctx^T, then out-projection.
"""

import sys

sys.path.insert(0, "/opt/trn_rl_repo")

from contextlib import ExitStack

import numpy as np
import ml_dtypes

BF = ml_dtypes.bfloat16

B, S, E, H, D = 4, 1024, 1024, 16, 64
TOK = B * S            # 4096
NCORES = 8
HPC = H // NCORES      # heads per core = 2
MAX_REL = 512
W = 1280               # Ppad row width (w = j - i + 640, w in [1, 1279] used)
WS = W + 1             # sheared row stride
BAND = 4               # |block_i - block_j| <= BAND handled via diagonal DMA
NC128 = S // 128       # 8 chunks per sequence

_CACHE = {}


def _build():
    import concourse.bacc as bacc
    import concourse.tile as tile
    from concourse import mybir
    from concourse.ap import AP

    F32 = mybir.dt.float32
    BF16 = mybir.dt.bfloat16
    EXP = mybir.ActivationFunctionType.Exp
    IDENT = mybir.ActivationFunctionType.Identity

    nc = bacc.Bacc(
        "TRN2", target_bir_lowering=False, debug=False, num_devices=NCORES
    )

    # ---------------- DRAM I/O ----------------
    qT_d = nc.dram_tensor("qT", [E, TOK], BF16, kind="ExternalInput")
    wq_d = nc.dram_tensor("wq", [E, 128], BF16, kind="ExternalInput")
    wk_d = nc.dram_tensor("wk", [E, 128], BF16, kind="ExternalInput")
    wv_d = nc.dram_tensor("wv", [E, 128], BF16, kind="ExternalInput")
    wo_d = nc.dram_tensor("wo", [128, E], BF16, kind="ExternalInput")
    bq_d = nc.dram_tensor("bq", [128, 1], F32, kind="ExternalInput")
    bk_d = nc.dram_tensor("bk", [128, 1], F32, kind="ExternalInput")
    bv_d = nc.dram_tensor("bv", [128, 1], F32, kind="ExternalInput")
    tt_d = nc.dram_tensor("ttT", [128, W], BF16, kind="ExternalInput")
    id_d = nc.dram_tensor("ident", [128, 128], BF16, kind="ExternalInput")
    out_d = nc.dram_tensor("outT", [E, TOK], BF16, kind="ExternalOutput")

    def split512(lo, hi):
        """split [lo,hi) at 512-grid lines (psum bank boundaries)"""
        if lo >= hi:
            return []
        cuts = [lo]
        g = (lo // 512 + 1) * 512
        while g < hi:
            cuts.append(g)
            g += 512
        cuts.append(hi)
        return list(zip(cuts[:-1], cuts[1:]))

    with tile.TileContext(nc) as tc, ExitStack() as ctx:
        const = ctx.enter_context(tc.tile_pool(name="const", bufs=1))
        big = ctx.enter_context(tc.tile_pool(name="bigsb", bufs=1))
        work = ctx.enter_context(tc.tile_pool(name="work", bufs=3))
        atp = ctx.enter_context(tc.tile_pool(name="atp", bufs=2))
        ctxp = ctx.enter_context(tc.tile_pool(name="ctxp", bufs=2))
        psA = ctx.enter_context(tc.tile_pool(name="psA", bufs=3, space="PSUM"))
        psB = ctx.enter_context(tc.tile_pool(name="psB", bufs=2, space="PSUM"))
        dram = ctx.enter_context(tc.tile_pool(name="dram", bufs=3, space="DRAM"))

        # ------------- load constants / inputs -------------
        qT = big.tile([128, 8, TOK], BF16, tag="qT")
        nc.sync.dma_start(qT[:], qT_d.ap().rearrange("(c p) t -> p c t", p=128))
        wq = const.tile([128, 8, 128], BF16, tag="wq")
        nc.sync.dma_start(wq[:], wq_d.ap().rearrange("(c p) m -> p c m", p=128))
        wk = const.tile([128, 8, 128], BF16, tag="wk")
        nc.sync.dma_start(wk[:], wk_d.ap().rearrange("(c p) m -> p c m", p=128))
        wv = const.tile([128, 8, 128], BF16, tag="wv")
        nc.sync.dma_start(wv[:], wv_d.ap().rearrange("(c p) m -> p c m", p=128))
        wo = const.tile([128, E], BF16, tag="wo")
        nc.sync.dma_start(wo[:], wo_d.ap())
        bq = const.tile([128, 1], F32, tag="bq")
        nc.sync.dma_start(bq[:], bq_d.ap())
        bk = const.tile([128, 1], F32, tag="bk")
        nc.sync.dma_start(bk[:], bk_d.ap())
        bv = const.tile([128, 1], F32, tag="bv")
        nc.sync.dma_start(bv[:], bv_d.ap())
        ttT = const.tile([128, W], BF16, tag="ttT")
        nc.sync.dma_start(ttT[:], tt_d.ap())
        ident = const.tile([128, 128], BF16, tag="ident")
        nc.sync.dma_start(ident[:], id_d.ap())
        onesF = const.tile([128, 64], F32, tag="onesF")
        nc.vector.memset(onesF[:], 1.0)

        QT = big.tile([128, TOK], BF16, tag="QT")
        KT = big.tile([128, TOK], BF16, tag="KT")
        V = big.tile([128, 32, 160], BF16, tag="V")
        nc.vector.memset(V[:, :, 64:65], 1.0)
        nc.vector.memset(V[:, :, 144:145], 1.0)

        # ------------- projections -------------
        # Q^T, K^T: [128(e_out), TOK] = W^T q^T, bias added via ACT
        for dst, wgt, bias in ((QT, wq, bq), (KT, wk, bk)):
            for t8 in range(8):
                ps = psA.tile([128, 512], F32, tag="big")
                for ec in range(8):
                    nc.tensor.matmul(
                        ps[:],
                        wgt[:, ec, :],
                        qT[:, ec, t8 * 512:(t8 + 1) * 512],
                        start=(ec == 0),
                        stop=(ec == 7),
                    )
                nc.scalar.activation(
                    dst[:, t8 * 512:(t8 + 1) * 512], ps[:], IDENT,
                    bias=bias[:], scale=1.0,
                )
        # V: project transposed like Q/K (cheap ldweights), then flip to
        # natural [tok, d] layout with two xbar DMA transposes per head.
        VT = big.tile([128, TOK], BF16, tag="VT")
        for t8 in range(8):
            ps = psA.tile([128, 512], F32, tag="big")
            for ec in range(8):
                nc.tensor.matmul(
                    ps[:], wv[:, ec, :], qT[:, ec, t8 * 512:(t8 + 1) * 512],
                    start=(ec == 0), stop=(ec == 7),
                )
            nc.scalar.activation(
                VT[:, t8 * 512:(t8 + 1) * 512], ps[:], IDENT,
                bias=bv[:], scale=1.0,
            )
        # bounce V^T through DRAM: DRAM-source xbar transposes avoid the
        # sb->sb-transpose hazard serialization (and read any row offset).
        vtd = dram.tile([128, TOK], BF16, tag="vtd")
        nc.sync.dma_start(vtd[:], VT[:])
        nc.sync.dma_start_transpose(V[:, :, 0:64], vtd[0:64, :])
        nc.scalar.dma_start_transpose(V[:, :, 80:144], vtd[64:128, :])

        # ------------- attention per (b, h) -------------
        # software pipeline: P-phase of (b,h) runs one step ahead of the
        # attention phase of the previous (b,h), keeping the PE busy while
        # the P DRAM round-trip for the current step completes.
        def emit_p_phase(b, h):
            t0 = b * S
            hr0, hr1 = h * 64, h * 64 + 64
            pd = dram.tile([S * WS], BF16, tag="pshear")
            fl = pd[:]
            edges_all = work.tile([128, 8, 2], F32, tag="edges")
            for icc in range(NC128):
                i0 = icc * 128
                lhs = QT[hr0:hr1, t0 + i0:t0 + i0 + 128]
                psP1 = psA.tile([128, 1024], F32, tag="big")
                psP2 = psA.tile([128, 256], F32, tag="big")
                for lo, hi in split512(0, 1024):
                    nc.tensor.matmul(psP1[:, lo:hi], lhs, ttT[hr0:hr1, lo:hi],
                                     start=True, stop=True)
                nc.tensor.matmul(psP2[:], lhs, ttT[hr0:hr1, 1024:W],
                                 start=True, stop=True)
                # clamp-edge columns (u=0 at w=128, u=1024 at w=1152)
                nc.vector.tensor_copy(edges_all[:, icc, 0:1], psP1[:, 128:129])
                nc.vector.tensor_copy(edges_all[:, icc, 1:2], psP2[:, 128:129])
                pp = work.tile([128, W], BF16, tag="ppad")
                if icc % 2 == 0:
                    nc.vector.tensor_copy(pp[:, 0:1024], psP1[:])
                    nc.scalar.copy(pp[:, 1024:W], psP2[:])
                else:
                    nc.scalar.copy(pp[:, 0:1024], psP1[:])
                    nc.vector.tensor_copy(pp[:, 1024:W], psP2[:])
                nc.gpsimd.dma_start(
                    AP(fl.tensor, fl.offset + i0 * WS, [(WS, 128), (1, W)]),
                    pp[:],
                )
            return fl, edges_all

        def emit_scores(b, h, fl, edges_all):
            t0 = b * S
            hr0, hr1 = h * 64, h * 64 + 64
            attnT = atp.tile([128, 8, S], BF16, tag="attnT")
            edt = dram.tile([NC128, 128, S], BF16, tag="expd")
            ed = [edt[i] for i in range(NC128)]
            for icc in range(NC128):
                i0 = icc * 128
                jlo = max(0, icc - BAND) * 128
                jhi = min(NC128, icc + BAND + 1) * 128
                jw = jhi - jlo

                bias_t = work.tile([128, 9 * 128], BF16, tag="bias")
                nc.gpsimd.dma_start(
                    bias_t[:, 0:jw],
                    AP(fl.tensor, fl.offset + i0 * W + jlo + W // 2,
                       [(W, 128), (1, jw)]),
                )

                ps = psA.tile([128, S], F32, tag="big")
                # QK first (start=True) so the PE never waits on the bias
                # DMA chain; the identity-matmul bias accumulates after.
                lhs = QT[hr0:hr1, t0 + i0:t0 + i0 + 128]
                for lo, hi in split512(0, S):
                    nc.tensor.matmul(
                        ps[:, lo:hi], lhs, KT[hr0:hr1, t0 + lo:t0 + hi],
                        start=True, stop=(lo >= jhi or hi <= jlo),
                    )
                for lo, hi in split512(jlo, jhi):
                    nc.tensor.matmul(
                        ps[:, lo:hi], ident[:], bias_t[:, lo - jlo:hi - jlo],
                        start=False, stop=True,
                    )

                ex = work.tile([128, S], BF16, tag="exp")
                if jlo > 0:
                    nc.scalar.activation(
                        ex[:, 0:jlo], ps[:, 0:jlo], EXP,
                        bias=edges_all[:, icc, 0:1], scale=1.0,
                    )
                nc.scalar.activation(
                    ex[:, jlo:jhi], ps[:, jlo:jhi], EXP, bias=0.0, scale=1.0
                )
                if jhi < S:
                    nc.scalar.activation(
                        ex[:, jhi:S], ps[:, jhi:S], EXP,
                        bias=edges_all[:, icc, 1:2], scale=1.0,
                    )
                # [i, j] -> [j, i] via DMA xbar, bounced through DRAM
                # (sb->sb xbar transposes are hazard-serialized against all
                # other sb->sb DMA traffic; DRAM-source xbars are not)
                eng2 = nc.scalar if icc % 2 == 0 else nc.sync
                eng3 = nc.sync if icc % 2 == 0 else nc.scalar
                eng2.dma_start(ed[icc], ex[:])
                eng3.dma_start_transpose(attnT[:, :, i0:i0 + 128], ed[icc])
            return attnT

        def emit_av_norm(b, h, attnT, ctxs):
            # ---- A@V + normalize, in two 512-column halves so the
            # reciprocal chain of one half overlaps the A@V of the next
            # (each half is a single psum bank; psB is double-buffered) ----
            for lo0 in (0, 512):
                hi0 = lo0 + 512
                psc = psB.tile([65, 512], F32, tag="ctx")
                for jc in range(NC128):
                    lhsv = V[:, b * 8 + jc, h * 80:h * 80 + 65]
                    nc.tensor.matmul(
                        psc[:], lhsv, attnT[:, jc, lo0:hi0],
                        start=(jc == 0), stop=(jc == 7),
                    )
                recS = work.tile([65, 512], F32, tag="recS")
                nc.vector.reciprocal(recS[64:65, :], psc[64:65, :])
                psr = psA.tile([64, 512], F32, tag="big")
                nc.tensor.matmul(psr[:], onesF[64:65, :], recS[64:65, :],
                                 start=True, stop=True)
                rbc = work.tile([64, 512], F32, tag="rbc")
                nc.vector.tensor_copy(rbc[:], psr[:])
                if h == 0:
                    nc.vector.tensor_mul(ctxs[0:64, lo0:hi0], psc[0:64, :], rbc[:])
                else:
                    th1 = work.tile([64, 512], BF16, tag="th1")
                    nc.vector.tensor_mul(th1[:], psc[0:64, :], rbc[:])
                    eng = nc.sync if lo0 == 0 else nc.scalar
                    eng.dma_start(ctxs[64:128, lo0:hi0], th1[:])

        def emit_outproj(b, ctxs):
            t0 = b * S
            for ec in range(8):
                pso = psA.tile([128, S], F32, tag="big")
                for lo, hi in split512(0, S):
                    nc.tensor.matmul(
                        pso[:, lo:hi], wo[:, ec * 128:(ec + 1) * 128],
                        ctxs[:, lo:hi], start=True, stop=True,
                    )
                ob = work.tile([128, S], BF16, tag="outsb")
                nc.scalar.copy(ob[:], pso[:])
                eng = nc.sync if ec % 2 == 0 else nc.scalar
                eng.dma_start(
                    out_d.ap()[ec * 128:(ec + 1) * 128, t0:t0 + S], ob[:]
                )

        phases = [(b, h) for b in range(B) for h in range(HPC)]
        ctxs_by_b = {}
        p_state = {}
        sc_state = {}

        def run_scores(bh):
            b, h = bh
            if h == 0:
                ctxs_by_b[b] = ctxp.tile([128, S], BF16, tag="ctxs",
                                         name=f"ctxs_{b}")
            fl, edges = p_state.pop(bh)
            sc_state[bh] = emit_scores(b, h, fl, edges)

        def run_av(bh):
            b, h = bh
            emit_av_norm(b, h, sc_state.pop(bh), ctxs_by_b[b])
            if h == 1:
                emit_outproj(b, ctxs_by_b.pop(b))

        for i, bh in enumerate(phases):
            p_state[bh] = emit_p_phase(*bh)
            if i >= 1:
                run_scores(phases[i - 1])
            if i >= 2:
                run_av(phases[i - 2])
        run_scores(phases[-1])
        run_av(phases[-2])
        run_av(phases[-1])

    nc.compile()
    return nc


def _host_prep(q, Wq, bq, Wk, bk, Wv, bv, Wo, bo, rel_table):
    x = np.ascontiguousarray(q.reshape(TOK, E).T).astype(BF)  # [E, TOK]
    ident = np.eye(128, dtype=BF)
    # padded/clamped rel table, transposed: ttT[d, w] = T[clip(w-128,0,1024), d]
    u = np.clip(np.arange(W) - 128, 0, 2 * MAX_REL)
    tt1 = np.ascontiguousarray(rel_table[u].T).astype(BF)  # [64, 1280]
    ttT = np.concatenate([tt1, tt1], axis=0)  # both partition halves
    maps = []
    for c in range(NCORES):
        sl = slice(c * 128, (c + 1) * 128)
        maps.append({
            "qT": x,
            "wq": Wq[:, sl].astype(BF),
            "wk": (Wk[:, sl] / 8.0).astype(BF),
            "wv": Wv[:, sl].astype(BF),
            "wo": Wo[sl, :].astype(BF),
            "bq": bq[sl].reshape(128, 1).astype(np.float32),
            "bk": (bk[sl] / 8.0).reshape(128, 1).astype(np.float32),
            "bv": bv[sl].reshape(128, 1).astype(np.float32),
            "ttT": ttT,
            "ident": ident,
        })
    return maps


def kernel(q, Wq, bq, Wk, bk, Wv, bv, Wo, bo, rel_table, _trace=False):
    from concourse.bass_utils import run_bass_kernel_spmd

    if "nc" not in _CACHE:
        _CACHE["nc"] = _build()
    nc = _CACHE["nc"]

    in_maps = _host_prep(q, Wq, bq, Wk, bk, Wv, bv, Wo, bo, rel_table)
    res = run_bass_kernel_spmd(
        nc, in_maps, list(range(NCORES)), trace=_trace
    )
    _CACHE["last_results"] = res
    acc = np.zeros((E, TOK), np.float32)
    for r in res.results:
        acc += np.asarray(r["outT"], dtype=np.float32)
    out = acc.T.reshape(B, S, E) + bo.astype(np.float32)
    return out.astype(np.float32)
